# revision 1
# baseline (speedup 1.0000x reference)
"""Mixed causal attention (B=8,L=1024,D=1024,H=16,NS=8) on 8 TRN2 cores.

Sharding: data-parallel over batch (core b owns batch b) for projections,
attention, out-proj.  The per-position (ns) projection weights are sharded
by position: core c computes q/k/v for position 1016+c for ALL batches
(reads only Wq_ns[c],Wk_ns[c],Wv_ns[c]), then an AllGather distributes the
3x[8,1024] results; each core extracts its batch's 8 tail rows with a
one-hot selection matmul (the program is SPMD-identical, so per-core row
selection is driven by a per-core input, not program structure).
"""

import sys
import os
from contextlib import ExitStack

import numpy as np

sys.path.insert(0, "/opt/trn_rl_repo")

import ml_dtypes  # noqa: E402
import concourse.bass as bass  # noqa: E402
import concourse.tile as tile  # noqa: E402
from concourse import bacc, mybir  # noqa: E402
from concourse._compat import with_exitstack  # noqa: E402
from concourse.bass_utils import run_bass_kernel_spmd  # noqa: E402

B, L, D, H, NS = 8, 1024, 1024, 16, 8
HD = D // H          # 64
LS = L - NS          # 1016
NCORES = 8
NEG = -1.0e9
BF = mybir.dt.bfloat16
F32 = mybir.dt.float32

_CACHE = {}
TRACE = False


@with_exitstack
def _attention_kernel(ctx: ExitStack, tc: tile.TileContext, aps: dict):
    nc = tc.nc

    sb = ctx.enter_context(tc.tile_pool(name="persist", bufs=1))
    wns_pool = ctx.enter_context(tc.tile_pool(name="wns", bufs=2))
    pt_pool = ctx.enter_context(tc.tile_pool(name="pt", bufs=2))
    stage = ctx.enter_context(tc.tile_pool(name="stage", bufs=2))
    ps = ctx.enter_context(tc.tile_pool(name="psum", bufs=1, space="PSUM"))
    psa = ctx.enter_context(tc.tile_pool(name="psacc", bufs=2, space="PSUM"))
    dram = ctx.enter_context(tc.tile_pool(name="dram", bufs=2, space="DRAM"))

    # ---- persistent SBUF tensors ----
    xT = sb.tile([128, 8 * 1024], BF)      # [d-part, dt*1024 + l]
    wq = sb.tile([128, 8 * 1024], BF)      # [d-part, dt*1024 + e]
    wk = sb.tile([128, 8 * 1024], BF)
    wv = sb.tile([128, 8 * 1024], BF)
    wo = sb.tile([128, 8 * 1024], BF)      # [e-part, et*1024 + e']
    qT = sb.tile([128, 8 * 1024], BF)      # [e-part, et*1024 + l]
    kT = sb.tile([128, 8 * 1024], BF)
    vb = sb.tile([128, 8 * 1040], BF)      # [l-part, lt*1040 + h*65 + eh]; col h*65+64 = ones
    oT = sb.tile([128, 8 * 1024], BF)      # [e-part, et*1024 + l]
    xtails = sb.tile([128, 64], BF)        # [d-part, dt*8 + bb]
    sel = sb.tile([64, 8], BF)             # one-hot row selector (per-core data)
    tri = sb.tile([128, 128], F32)         # tri[p,f] = 0 if p<=f else NEG
    mbias = sb.tile([128, 8], F32)         # key-padding additive bias per k-block
    ones1 = sb.tile([1, 128], BF)
    nsb = sb.tile([8, 3072], BF)           # my position's q|k|v for all batches
    fullg = sb.tile([64, 3072], BF)        # gathered: row n*8+bb

    # ---- input DMAs ----
    for dt in range(8):
        r = slice(dt * 128, dt * 128 + 128)
        nc.gpsimd.dma_start(xT[:, bass.ts(dt, 1024)], aps["xT"][r, :])
        nc.gpsimd.dma_start(wq[:, bass.ts(dt, 1024)], aps["wqT"][r, :])
        nc.gpsimd.dma_start(wk[:, bass.ts(dt, 1024)], aps["wkT"][r, :])
        nc.gpsimd.dma_start(wv[:, bass.ts(dt, 1024)], aps["wvT"][r, :])
        nc.gpsimd.dma_start(wo[:, bass.ts(dt, 1024)], aps["woutT"][r, :])
        nc.gpsimd.dma_start(xtails[:, bass.ts(dt, 8)], aps["xtails"][r, :])
    nc.gpsimd.dma_start(sel[:], aps["sel"][:])
    nc.gpsimd.dma_start(tri[:], aps["tri"][:])
    nc.gpsimd.dma_start(mbias[:], aps["maskbias"][:])
    nc.gpsimd.dma_start(ones1[:], aps["onesb"][:])

    # ---- phase 1: ns projections for my position (all batches) ----
    nsacc = sb.tile([8, 3072], F32)
    for dt in range(8):
        wt = wns_pool.tile([128, 3072], BF)
        nc.gpsimd.dma_start(wt[:], aps["wnsT"][dt * 128:dt * 128 + 128, :])
        for ck in range(6):
            pp = ps.tile([8, 512], F32, name="nsp", bufs=1)
            nc.tensor.matmul(
                pp[:],
                xtails[:, bass.ts(dt, 8)],
                wt[:, bass.ts(ck, 512)],
                start=True,
                stop=True,
            )
            if dt == 0:
                nc.vector.tensor_copy(nsacc[:, bass.ts(ck, 512)], pp[:])
            else:
                nc.vector.tensor_tensor(
                    nsacc[:, bass.ts(ck, 512)],
                    nsacc[:, bass.ts(ck, 512)],
                    pp[:],
                    mybir.AluOpType.add,
                )
    for ck in range(6):
        nc.vector.tensor_copy(nsb[:, bass.ts(ck, 512)], nsacc[:, bass.ts(ck, 512)])

    gin = dram.tile([8, 3072], BF)
    gout = dram.tile([64, 3072], BF)
    nc.gpsimd.dma_start(gin[:], nsb[:])
    nc.gpsimd.collective_compute(
        "AllGather",
        mybir.AluOpType.bypass,
        replica_groups=[list(range(NCORES))],
        ins=[gin.opt()],
        outs=[gout.opt()],
    )
    nc.gpsimd.dma_start(fullg[:], gout[:])

    # ---- phase 2: shared Q/K/V projections ----
    for w, dst in ((wq, qT), (wk, kT)):
        for et in range(8):
            for lg in range(2):
                acc = ps.tile([128, 512], F32, name="acc", bufs=2)
                for dt in range(8):
                    nc.tensor.matmul(
                        acc[:],
                        w[:, dt * 1024 + et * 128: dt * 1024 + et * 128 + 128],
                        xT[:, dt * 1024 + lg * 512: dt * 1024 + lg * 512 + 512],
                        start=(dt == 0),
                        stop=(dt == 7),
                    )
                nc.vector.tensor_copy(
                    dst[:, et * 1024 + lg * 512: et * 1024 + lg * 512 + 512], acc[:]
                )

    # V in natural [l, e] layout, interleaved with a ones column per head
    for lt in range(8):
        for hh in range(16):
            nc.vector.memset(vb[:, lt * 1040 + hh * 65 + 64: lt * 1040 + hh * 65 + 65], 1.0)
        for eg in range(2):
            acc = ps.tile([128, 512], F32, name="acc", bufs=2)
            for dt in range(8):
                nc.tensor.matmul(
                    acc[:],
                    xT[:, dt * 1024 + lt * 128: dt * 1024 + lt * 128 + 128],
                    wv[:, dt * 1024 + eg * 512: dt * 1024 + eg * 512 + 512],
                    start=(dt == 0),
                    stop=(dt == 7),
                )
            for hh in range(8):
                h = eg * 8 + hh
                nc.vector.tensor_copy(
                    vb[:, lt * 1040 + h * 65: lt * 1040 + h * 65 + 64],
                    acc[:, hh * 64: hh * 64 + 64],
                )

    # ---- phase 3: overwrite tail rows (1016..1023) from gathered ns results ----
    # q/k tails, transposed via selection matmul: lhsT=fullg chunk [64,128], rhs=sel [64,8]
    for c2 in range(16):
        tp = ps.tile([128, 512], F32, name="nsp", bufs=1)
        nc.tensor.matmul(
            tp[:, 0:8],
            fullg[:, c2 * 128: c2 * 128 + 128],
            sel[:],
            start=True,
            stop=True,
        )
        dst = qT if c2 < 8 else kT
        et = c2 % 8
        nc.vector.tensor_copy(dst[:, et * 1024 + 1016: et * 1024 + 1024], tp[:, 0:8])
    # v tails natural: lhsT=sel [64,8], rhs=fullg v cols
    for vg in range(2):
        tp = ps.tile([128, 512], F32, name="nsp", bufs=1)
        nc.tensor.matmul(
            tp[0:8, :],
            sel[:],
            fullg[:, 2048 + vg * 512: 2048 + vg * 512 + 512],
            start=True,
            stop=True,
        )
        vt = stage.tile([8, 512], BF, name="rec16")
        nc.vector.tensor_copy(vt[:], tp[0:8, :])
        for hh in range(8):
            h = vg * 8 + hh
            nc.gpsimd.dma_start(
                vb[120:128, 7 * 1040 + h * 65: 7 * 1040 + h * 65 + 64],
                vt[:, hh * 64: hh * 64 + 64],
            )

    # ---- phase 4: causal attention per head ----
    for h in range(H):
        et, r0 = h // 2, (h % 2) * 64
        for g in range(2):
            nj = 4 * g + 4
            pt = pt_pool.tile([128, 8 * 512], BF)
            for j in range(nj):
                lead = j * 128 - g * 512
                if lead > 0:
                    nc.vector.memset(pt[:, j * 512: j * 512 + lead], 0.0)
            for j in range(nj):
                sp = ps.tile([128, 512], F32, name="sp", bufs=2)
                nc.tensor.matmul(
                    sp[:],
                    kT[r0:r0 + 64, et * 1024 + j * 128: et * 1024 + j * 128 + 128],
                    qT[r0:r0 + 64, et * 1024 + g * 512: et * 1024 + g * 512 + 512],
                    start=True,
                    stop=True,
                )
                bias = mbias[:, j: j + 1]
                lead = max(0, j * 128 - g * 512)
                if j >= 4 * g:  # diagonal block lives in this q-group
                    nc.vector.tensor_tensor(
                        sp[:, lead: lead + 128],
                        sp[:, lead: lead + 128],
                        tri[:],
                        mybir.AluOpType.add,
                    )
                nc.scalar.activation(
                    pt[:, j * 512 + lead: j * 512 + 512],
                    sp[:, lead:512],
                    mybir.ActivationFunctionType.Exp,
                    bias=bias,
                    scale=0.125,
                )
            op = psa.tile([128, 512], F32)
            for j in range(nj):
                nc.tensor.matmul(
                    op[0:65, :],
                    vb[:, j * 1040 + h * 65: j * 1040 + h * 65 + 65],
                    pt[:, j * 512: j * 512 + 512],
                    start=(j == 0),
                    stop=(j == nj - 1),
                )
            rec32 = stage.tile([1, 512], F32)
            rec16 = stage.tile([1, 512], BF)
            nc.vector.reciprocal(rec32[:], op[64:65, :])
            nc.vector.tensor_copy(rec16[:], rec32[:])
            bp = ps.tile([128, 512], F32, name="bp", bufs=1)
            nc.tensor.matmul(bp[:], ones1[:], rec16[:], start=True, stop=True)
            bb = stage.tile([128, 512], F32, name="ys")
            nc.scalar.activation(
                bb[0:64, :], bp[0:64, :], mybir.ActivationFunctionType.Copy, scale=1.0
            )
            nc.vector.tensor_tensor(
                oT[r0:r0 + 64, et * 1024 + g * 512: et * 1024 + g * 512 + 512],
                op[0:64, :],
                bb[0:64, :],
                mybir.AluOpType.mult,
            )

    # ---- phase 5: output projection, natural [l, e'] ----
    for lt in range(8):
        for eg in range(2):
            acc = ps.tile([128, 512], F32, name="acc", bufs=2)
            for et in range(8):
                nc.tensor.matmul(
                    acc[:],
                    oT[:, et * 1024 + lt * 128: et * 1024 + lt * 128 + 128],
                    wo[:, et * 1024 + eg * 512: et * 1024 + eg * 512 + 512],
                    start=(et == 0),
                    stop=(et == 7),
                )
            ys = stage.tile([128, 512], F32)
            nc.vector.tensor_copy(ys[:], acc[:])
            nc.gpsimd.dma_start(
                aps["y"][lt * 128: lt * 128 + 128, eg * 512: eg * 512 + 512], ys[:]
            )


def _build():
    if "nc" in _CACHE:
        return _CACHE["nc"]
    nc = bacc.Bacc("TRN2", target_bir_lowering=False, debug=False, num_devices=NCORES)
    aps = {}
    for name, shape, dt in [
        ("xT", [1024, 1024], BF),
        ("wqT", [1024, 1024], BF),
        ("wkT", [1024, 1024], BF),
        ("wvT", [1024, 1024], BF),
        ("woutT", [1024, 1024], BF),
        ("wnsT", [1024, 3072], BF),
        ("xtails", [1024, 8], BF),
        ("sel", [64, 8], BF),
        ("tri", [128, 128], F32),
        ("maskbias", [128, 8], F32),
        ("onesb", [1, 128], BF),
    ]:
        aps[name] = nc.dram_tensor(name, shape, dt, kind="ExternalInput").ap()
    aps["y"] = nc.dram_tensor("y", [1024, 1024], F32, kind="ExternalOutput").ap()

    with tile.TileContext(nc) as tc:
        _attention_kernel(tc, aps)
    nc.compile()
    _CACHE["nc"] = nc
    return nc


def kernel(x, key_padding_mask, Wq_s, Wk_s, Wv_s, Wq_ns, Wk_ns, Wv_ns, W_out, **kw):
    x = np.asarray(x, np.float32)
    mask = np.asarray(key_padding_mask)
    bf = ml_dtypes.bfloat16

    wqT = np.ascontiguousarray(np.asarray(Wq_s, np.float32).T.astype(bf))
    wkT = np.ascontiguousarray(np.asarray(Wk_s, np.float32).T.astype(bf))
    wvT = np.ascontiguousarray(np.asarray(Wv_s, np.float32).T.astype(bf))
    woT = np.ascontiguousarray(np.asarray(W_out, np.float32).T.astype(bf))
    tri = np.where(
        np.arange(128)[:, None] <= np.arange(128)[None, :], 0.0, NEG
    ).astype(np.float32)
    onesb = np.ones((1, 128), bf)

    Wq_ns = np.asarray(Wq_ns, np.float32)
    Wk_ns = np.asarray(Wk_ns, np.float32)
    Wv_ns = np.asarray(Wv_ns, np.float32)

    in_maps = []
    for c in range(NCORES):
        xT = np.ascontiguousarray(x[c].T.astype(bf))
        xtails = np.ascontiguousarray(x[:, LS + c, :].T.astype(bf))  # [1024 d, 8 bb]
        wnsT = np.ascontiguousarray(
            np.concatenate([Wq_ns[c].T, Wk_ns[c].T, Wv_ns[c].T], axis=1).astype(bf)
        )
        selm = np.zeros((64, 8), bf)
        for n in range(NS):
            selm[n * 8 + c, n] = 1.0
        mb = np.where(mask[c], 0.0, NEG).astype(np.float32).reshape(8, 128).T
        mb = np.ascontiguousarray(mb)
        in_maps.append(
            {
                "xT": xT,
                "wqT": wqT,
                "wkT": wkT,
                "wvT": wvT,
                "woutT": woT,
                "wnsT": wnsT,
                "xtails": xtails,
                "sel": selm,
                "tri": tri,
                "maskbias": mb,
                "onesb": onesb,
            }
        )

    nc = _build()
    res = run_bass_kernel_spmd(nc, in_maps, list(range(NCORES)), trace=TRACE)
    _CACHE["exec_time_ns"] = res.exec_time_ns
    _CACHE["res"] = res
    out = np.stack([res.results[c]["y"] for c in range(NCORES)], axis=0)
    return out.astype(np.float32)



# revision 11
# speedup vs baseline: 1.2294x; 1.2294x over previous
"""Mixed causal attention (B=8,L=1024,D=1024,H=16,NS=8) on 8 TRN2 cores.

Sharding: data-parallel over batch (core b owns batch b).  The per-position
(ns) projection weights are sharded by position: core c computes q/k/v for
position 1016+c for ALL batches, an AllGather distributes the results, and
each core extracts its batch's 8 tail rows with a one-hot selection matmul.

Pipeline (engine-balanced, keeps PE dense so HAM stays at full clock):
  A: V projection (psum-accumulated, strided interleave copy into vb)
  B: ns projections (6 psum banks accumulate over dt) -> AllGather (async)
  C: per head-pair i: QK proj strip et=i overlapped with g0 attention of
     pair i-1 (scores row-tiled 2-up, full-width exps, batched causal-mask
     multiply, fast reciprocal, fp32r broadcast matmul)
  T: tail merge (q/k/v rows 1016..1023 from gathered ns results)
  D: per pair i: g1 scores of pair i overlapped with AV of pair i-1 and
     one out-projection chunk (lt<4 columns complete after C)
  E: remaining out-projection chunks
"""

import sys
from contextlib import ExitStack

import numpy as np

sys.path.insert(0, "/opt/trn_rl_repo")

import ml_dtypes  # noqa: E402
import concourse.bass as bass  # noqa: E402
import concourse.tile as tile  # noqa: E402
from concourse import bacc, mybir  # noqa: E402
from concourse._compat import with_exitstack  # noqa: E402
from concourse.bass_utils import run_bass_kernel_spmd  # noqa: E402

B, L, D, H, NS = 8, 1024, 1024, 16, 8
HD = D // H          # 64
LS = L - NS          # 1016
NCORES = 8
NEG = -1.0e9
BF = mybir.dt.bfloat16
F32 = mybir.dt.float32
F32R = mybir.dt.float32r

_CACHE = {}
TRACE = False
DEBUG = False


@with_exitstack
def _attention_kernel(ctx: ExitStack, tc: tile.TileContext, aps: dict):
    nc = tc.nc
    Exp = mybir.ActivationFunctionType.Exp
    MUL = mybir.AluOpType.mult

    sb = ctx.enter_context(tc.tile_pool(name="persist", bufs=1))
    wpool = ctx.enter_context(tc.tile_pool(name="wpool", bufs=1))
    wns_pool = ctx.enter_context(tc.tile_pool(name="wns", bufs=2))
    stage = ctx.enter_context(tc.tile_pool(name="stage", bufs=3))
    accp = ctx.enter_context(tc.tile_pool(name="accp", bufs=2, space="PSUM"))
    dbgp = ctx.enter_context(tc.tile_pool(name="dbgp", bufs=1)) if DEBUG else None
    dram = ctx.enter_context(tc.tile_pool(name="dram", bufs=2, space="DRAM"))

    # ---- persistent SBUF tensors ----
    xT = sb.tile([128, 8 * 1024], BF)      # [d-part, dt*1024 + l]
    qT = sb.tile([128, 8 * 1024], BF)      # [e-part, et*1024 + l]
    kT = sb.tile([128, 8 * 1024], BF)
    vb = sb.tile([128, 8 * 1040], BF)      # [l-part, lt*1040 + h*65 + eh]; col h*65+64 = ones
    oT = sb.tile([128, 8 * 1024], BF)      # [e-part, et*1024 + l]
    xtails = sb.tile([128, 64], BF)        # [d-part, dt*8 + bb]
    sel = sb.tile([64, 8], BF)             # one-hot row selector (per-core data)
    mbias = sb.tile([128, 8], F32)         # key-padding additive bias per k-block
    cm0 = sb.tile([128, 2048], BF)         # causal 0/1 mask, g=0 (j=0..3)
    cm1 = sb.tile([128, 2048], BF)         # causal 0/1 mask, g=1 (j=4..7)
    ones1 = sb.tile([1, 64], BF)
    nsb = sb.tile([8, 3072], BF)           # my position's q|k|v for all batches
    fullg = sb.tile([64, 3072], BF)        # gathered: row n*8+bb
    # per-head exp'd score tiles (bufs=1; subtile deps pipeline across pairs)
    pt0 = [sb.tile([128, 2048], BF, name=f"pt0_{i}") for i in range(2)]  # g=0
    pt1 = [sb.tile([128, 4096], BF, name=f"pt1_{i}") for i in range(2)]  # g=1

    wq = sb.tile([128, 8 * 1024], BF)
    wk = sb.tile([128, 8 * 1024], BF)
    wv = wpool.tile([128, 8 * 1024], BF, name="wslot")

    # ---- input DMAs (batched 3D APs, spread across engine queues) ----
    def load_w(eng, dst, name):
        d = dst[:].rearrange("p (t l) -> p t l", l=1024)
        s = aps[name][:, :].rearrange("(t p) l -> p t l", p=128)
        eng.dma_start(d, s)

    load_w(nc.sync, xT, "xT")
    load_w(nc.sync, wv, "wvT")
    load_w(nc.scalar, wq, "wqT")
    load_w(nc.scalar, wk, "wkT")
    nc.gpsimd.dma_start(
        xtails[:].rearrange("p (t b) -> p t b", b=8),
        aps["xtails"][:, :].rearrange("(t p) b -> p t b", p=128),
    )
    nc.gpsimd.dma_start(sel[:], aps["sel"][:])
    nc.sync.dma_start(mbias[:], aps["maskbias"][:])
    nc.sync.dma_start(cm0[:], aps["cmask0"][:])
    nc.sync.dma_start(cm1[:], aps["cmask1"][:])
    nc.sync.dma_start(ones1[:], aps["onesb"][:])

    # ones columns of vb (col h*65+64 per lt): one strided memset
    nc.vector.memset(
        vb[:].rearrange("p (t h c) -> p t h c", h=16, c=65)[:, :, :, 64:65], 1.0
    )

    # ---- phase A: V projection, natural [l, e] layout interleaved per head ----
    def v_proj(eg):
        for lt in range(8):
            acc = accp.tile([128, 512], F32, name="acc")
            for dt in range(8):
                nc.tensor.matmul(
                    acc[:],
                    xT[:, dt * 1024 + lt * 128: dt * 1024 + lt * 128 + 128],
                    wv[:, dt * 1024 + eg * 512: dt * 1024 + eg * 512 + 512],
                    start=(dt == 0),
                    stop=(dt == 7),
                )
            base = lt * 1040 + eg * 520
            dst = vb[:, base: base + 520].rearrange("p (h c) -> p h c", c=65)[:, :, 0:64]
            src = acc[:].rearrange("p (h c) -> p h c", c=64)
            nc.vector.tensor_copy(dst, src)

    v_proj(0)

    # ---- phase B: ns projections for my position (all batches) ----
    with tc.tile_pool(name="nsps", bufs=1, space="PSUM") as nsps:
        nspp = [nsps.tile([8, 512], F32, name=f"nsp{ck}") for ck in range(6)]
        for dt in range(8):
            wt = wns_pool.tile([128, 3072], BF)
            nc.gpsimd.dma_start(wt[:], aps["wnsT"][dt * 128:dt * 128 + 128, :])
            for ck in range(6):
                nc.tensor.matmul(
                    nspp[ck][:],
                    xtails[:, bass.ts(dt, 8)],
                    wt[:, bass.ts(ck, 512)],
                    start=(dt == 0),
                    stop=(dt == 7),
                )
        for ck in range(6):
            nc.vector.tensor_copy(nsb[:, bass.ts(ck, 512)], nspp[ck][:])

    gin = dram.tile([8, 3072], BF)
    gout = dram.tile([64, 3072], BF)
    nc.gpsimd.dma_start(gin[:], nsb[:])
    nc.gpsimd.collective_compute(
        "AllGather",
        mybir.AluOpType.bypass,
        replica_groups=[list(range(NCORES))],
        ins=[gin.opt()],
        outs=[gout.opt()],
    )
    nc.gpsimd.dma_start(fullg[:], gout[:])

    v_proj(1)

    # wo reuses wv's buffer (pool bufs=3); DMA starts once V is done with wv
    wo = wpool.tile([128, 8 * 1024], BF, name="wslot")
    load_w(nc.gpsimd, wo, "woutT")

    spp = ctx.enter_context(tc.tile_pool(name="spp", bufs=4, space="PSUM"))
    opp = ctx.enter_context(tc.tile_pool(name="opp", bufs=2, space="PSUM"))

    def qk_proj(et):
        for w, dst in ((wq, qT), (wk, kT)):
            for lg in range(2):
                acc = accp.tile([128, 512], F32, name="acc")
                for dt in range(8):
                    nc.tensor.matmul(
                        acc[:],
                        w[:, dt * 1024 + et * 128: dt * 1024 + et * 128 + 128],
                        xT[:, dt * 1024 + lg * 512: dt * 1024 + lg * 512 + 512],
                        start=(dt == 0),
                        stop=(dt == 7),
                    )
                nc.vector.tensor_copy(
                    dst[:, et * 1024 + lg * 512: et * 1024 + lg * 512 + 512], acc[:]
                )

    def attn(et, g):
        """Scores (row-tiled 2-up) + exp + mask + AV + normalize for pair et."""
        nj = 4 * g + 4
        pts = pt0 if g == 0 else pt1
        cm = cm0 if g == 0 else cm1
        qs = et * 1024 + g * 512
        # scores + exp, both heads interleaved per j (concurrent row tiles)
        sps = {}
        for j in range(nj):
            for half in range(2):
                r0 = half * 64
                sp = spp.tile([128, 512], F32, name="sp")
                nc.tensor.matmul(
                    sp[:],
                    kT[r0:r0 + 64, et * 1024 + j * 128: et * 1024 + j * 128 + 128],
                    qT[r0:r0 + 64, qs: qs + 512],
                    start=True,
                    stop=True,
                )
                sps[(j, half)] = sp
        for j in range(nj):
            for half in range(2):
                nc.scalar.activation(
                    pts[half][:, j * 512: j * 512 + 512],
                    sps[(j, half)][:],
                    Exp,
                    bias=mbias[:, j: j + 1],
                    scale=0.125,
                )
        # causal mask: zero strictly-upper part of diagonal-containing chunks
        mw = 2048
        moff = 0 if g == 0 else 2048
        for half in range(2):
            nc.vector.tensor_tensor(
                pts[half][:, moff: moff + mw],
                pts[half][:, moff: moff + mw],
                cm[:],
                MUL,
            )
        # AV (ones column in vb row 64 gives the softmax denominator)
        for half in range(2):
            h = 2 * et + half
            op = opp.tile([128, 512], F32, name="op")
            for j in range(nj):
                nc.tensor.matmul(
                    op[0:65, :],
                    vb[:, j * 1040 + h * 65: j * 1040 + h * 65 + 65],
                    pts[half][:, j * 512: j * 512 + 512],
                    start=(j == 0),
                    stop=(j == nj - 1),
                )
            den = stage.tile([1, 512], F32, name="den", bufs=2)
            nc.vector.tensor_copy(den[:], op[64:65, :])
            rec = stage.tile([1, 512], F32, name="rec", bufs=2)
            nc.vector.reciprocal_approx_fast(rec[:], den[:])
            rec16 = stage.tile([1, 512], BF, name="rec16", bufs=2)
            nc.vector.tensor_copy(rec16[:], rec[:])
            bp = spp.tile([128, 512], F32, name="sp")
            nc.tensor.matmul(
                bp[0:64, :],
                ones1[:],
                rec16[:],
                start=True,
                stop=True,
            )
            bb = stage.tile([64, 512], BF, name="bb")
            nc.vector.tensor_copy(bb[:], bp[0:64, :])
            if DEBUG and et == 0 and g == 0 and half == 0:
                dop = dbgp.tile([65, 512], F32, name="dop")
                nc.vector.tensor_copy(dop[:], op[0:65, :])
                nc.gpsimd.dma_start(aps["dbg_op"][:, :], dop[:])
                nc.gpsimd.dma_start(aps["dbg_rec"][:, :], rec[:])
                nc.gpsimd.dma_start(aps["dbg_bb"][:, :], bb[:])
            nc.vector.tensor_tensor(
                oT[half * 64: half * 64 + 64, qs: qs + 512],
                op[0:64, :],
                bb[:],
                MUL,
            )

    def out_proj(lt, eg):
        acc = accp.tile([128, 512], F32, name="acc")
        for et in range(8):
            nc.tensor.matmul(
                acc[:],
                oT[:, et * 1024 + lt * 128: et * 1024 + lt * 128 + 128],
                wo[:, et * 1024 + eg * 512: et * 1024 + eg * 512 + 512],
                start=(et == 0),
                stop=(et == 7),
            )
        ys = stage.tile([128, 512], F32, name="ys")
        nc.vector.tensor_copy(ys[:], acc[:])
        nc.gpsimd.dma_start(
            aps["y"][lt * 128: lt * 128 + 128, eg * 512: eg * 512 + 512], ys[:]
        )

    # ---- phase C: QK projection pipelined with g0 attention (lag 1) ----
    for i in range(9):
        if i < 8:
            qk_proj(i)
        if i >= 1:
            attn(i - 1, 0)

    # ---- phase T: merge tail rows (1016..1023) from gathered ns results ----
    for c2 in range(16):
        tp = spp.tile([128, 512], F32, name="sp")
        nc.tensor.matmul(
            tp[:, 0:8],
            fullg[:, c2 * 128: c2 * 128 + 128],
            sel[:],
            start=True,
            stop=True,
        )
        dst = qT if c2 < 8 else kT
        et = c2 % 8
        nc.vector.tensor_copy(dst[:, et * 1024 + 1016: et * 1024 + 1024], tp[:, 0:8])
    for vg in range(2):
        tp = spp.tile([128, 512], F32, name="sp")
        nc.tensor.matmul(
            tp[0:8, :],
            sel[:],
            fullg[:, 2048 + vg * 512: 2048 + vg * 512 + 512],
            start=True,
            stop=True,
        )
        vt = stage.tile([8, 512], BF, name="vt", bufs=1)
        nc.vector.tensor_copy(vt[:], tp[0:8, :])
        dst = (
            vb[120:128, 7 * 1040 + vg * 520: 7 * 1040 + vg * 520 + 520]
            .rearrange("p (h c) -> p h c", c=65)[:, :, 0:64]
        )
        nc.gpsimd.dma_start(dst, vt[:].rearrange("p (h c) -> p h c", c=64))

    # ---- phase D: g1 attention pipelined with out-proj of finished columns ----
    for i in range(8):
        attn(i, 1)
        if i >= 1:
            out_proj((i - 1) // 2, (i - 1) % 2)

    # ---- phase E: remaining output projection ----
    out_proj(3, 1)
    for lt in range(4, 8):
        for eg in range(2):
            out_proj(lt, eg)

    if DEBUG:
        nc.gpsimd.dma_start(aps["dbg_qT"][:, :], qT[:])
        nc.gpsimd.dma_start(aps["dbg_kT"][:, :], kT[:])
        nc.gpsimd.dma_start(aps["dbg_vb"][:, :], vb[:])
        nc.gpsimd.dma_start(aps["dbg_oT"][:, :], oT[:])
        nc.gpsimd.dma_start(aps["dbg_pt0"][:, :], pt0[0][:])


def _build():
    if "nc" in _CACHE:
        return _CACHE["nc"]
    nc = bacc.Bacc("TRN2", target_bir_lowering=False, debug=False, num_devices=NCORES)
    aps = {}
    for name, shape, dt in [
        ("xT", [1024, 1024], BF),
        ("wqT", [1024, 1024], BF),
        ("wkT", [1024, 1024], BF),
        ("wvT", [1024, 1024], BF),
        ("woutT", [1024, 1024], BF),
        ("wnsT", [1024, 3072], BF),
        ("xtails", [1024, 8], BF),
        ("sel", [64, 8], BF),
        ("maskbias", [128, 8], F32),
        ("cmask0", [128, 2048], BF),
        ("cmask1", [128, 2048], BF),
        ("onesb", [1, 64], BF),
    ]:
        aps[name] = nc.dram_tensor(name, shape, dt, kind="ExternalInput").ap()
    aps["y"] = nc.dram_tensor("y", [1024, 1024], F32, kind="ExternalOutput").ap()
    if DEBUG:
        for name, shape, dt in [
            ("dbg_op", [65, 512], F32),
            ("dbg_rec", [1, 512], F32),
            ("dbg_bb", [64, 512], BF),
            ("dbg_qT", [128, 8192], BF),
            ("dbg_kT", [128, 8192], BF),
            ("dbg_vb", [128, 8320], BF),
            ("dbg_oT", [128, 8192], BF),
            ("dbg_pt0", [128, 2048], BF),
        ]:
            aps[name] = nc.dram_tensor(name, shape, dt, kind="ExternalOutput").ap()

    with tile.TileContext(nc) as tc:
        _attention_kernel(tc, aps)
    nc.compile()
    _CACHE["nc"] = nc
    return nc


def kernel(x, key_padding_mask, Wq_s, Wk_s, Wv_s, Wq_ns, Wk_ns, Wv_ns, W_out, **kw):
    x = np.asarray(x, np.float32)
    mask = np.asarray(key_padding_mask)
    bf = ml_dtypes.bfloat16

    wqT = np.ascontiguousarray(np.asarray(Wq_s, np.float32).T.astype(bf))
    wkT = np.ascontiguousarray(np.asarray(Wk_s, np.float32).T.astype(bf))
    wvT = np.ascontiguousarray(np.asarray(Wv_s, np.float32).T.astype(bf))
    woT = np.ascontiguousarray(np.asarray(W_out, np.float32).T.astype(bf))
    onesb = np.ones((1, 64), bf)

    # causal 0/1 masks: cm[g][p, jj*512+qq] = (j*128+p <= g*512+qq), j = 4g+jj
    p = np.arange(128)[:, None]
    qq = np.arange(512)[None, :]
    cm = []
    for g in range(2):
        blocks = [
            ((4 * g + jj) * 128 + p <= g * 512 + qq).astype(bf) for jj in range(4)
        ]
        cm.append(np.ascontiguousarray(np.concatenate(blocks, axis=1)))

    Wq_ns = np.asarray(Wq_ns, np.float32)
    Wk_ns = np.asarray(Wk_ns, np.float32)
    Wv_ns = np.asarray(Wv_ns, np.float32)

    in_maps = []
    for c in range(NCORES):
        xT = np.ascontiguousarray(x[c].T.astype(bf))
        xtails = np.ascontiguousarray(x[:, LS + c, :].T.astype(bf))  # [1024 d, 8 bb]
        wnsT = np.ascontiguousarray(
            np.concatenate([Wq_ns[c].T, Wk_ns[c].T, Wv_ns[c].T], axis=1).astype(bf)
        )
        selm = np.zeros((64, 8), bf)
        for n in range(NS):
            selm[n * 8 + c, n] = 1.0
        mb = np.where(mask[c], 0.0, NEG).astype(np.float32).reshape(8, 128).T
        mb = np.ascontiguousarray(mb)
        in_maps.append(
            {
                "xT": xT,
                "wqT": wqT,
                "wkT": wkT,
                "wvT": wvT,
                "woutT": woT,
                "wnsT": wnsT,
                "xtails": xtails,
                "sel": selm,
                "maskbias": mb,
                "cmask0": cm[0],
                "cmask1": cm[1],
                "onesb": onesb,
            }
        )

    nc = _build()
    res = run_bass_kernel_spmd(nc, in_maps, list(range(NCORES)), trace=TRACE)
    _CACHE["exec_time_ns"] = res.exec_time_ns
    _CACHE["res"] = res
    out = np.stack([res.results[c]["y"] for c in range(NCORES)], axis=0)
    return out.astype(np.float32)


# revision 12
# speedup vs baseline: 1.2882x; 1.0478x over previous
"""Mixed causal attention (B=8,L=1024,D=1024,H=16,NS=8) on 8 TRN2 cores.

Sharding: data-parallel over batch (core b owns batch b).  The per-position
(ns) projection weights are sharded by position: core c computes q/k/v for
position 1016+c for ALL batches, an AllGather distributes the results, and
each core extracts its batch's 8 tail rows with a one-hot selection matmul.

Pipeline (engine-balanced, keeps PE dense so HAM stays at full clock):
  A: V projection (psum-accumulated, strided interleave copy into vb)
  B: ns projections (6 psum banks accumulate over dt) -> AllGather (async)
  C: per head-pair i: QK proj strip et=i overlapped with g0 attention of
     pair i-1 (scores row-tiled 2-up, full-width exps, batched causal-mask
     multiply, fast reciprocal, fp32r broadcast matmul)
  T: tail merge (q/k/v rows 1016..1023 from gathered ns results)
  D: per pair i: g1 scores of pair i overlapped with AV of pair i-1 and
     one out-projection chunk (lt<4 columns complete after C)
  E: remaining out-projection chunks
"""

import sys
from contextlib import ExitStack

import numpy as np

sys.path.insert(0, "/opt/trn_rl_repo")

import ml_dtypes  # noqa: E402
import concourse.bass as bass  # noqa: E402
import concourse.tile as tile  # noqa: E402
from concourse import bacc, mybir  # noqa: E402
from concourse._compat import with_exitstack  # noqa: E402
from concourse.bass_utils import run_bass_kernel_spmd  # noqa: E402

B, L, D, H, NS = 8, 1024, 1024, 16, 8
HD = D // H          # 64
LS = L - NS          # 1016
NCORES = 8
NEG = -1.0e9
BF = mybir.dt.bfloat16
F32 = mybir.dt.float32
F32R = mybir.dt.float32r

_CACHE = {}
TRACE = False
DEBUG = False


@with_exitstack
def _attention_kernel(ctx: ExitStack, tc: tile.TileContext, aps: dict):
    nc = tc.nc
    Exp = mybir.ActivationFunctionType.Exp
    MUL = mybir.AluOpType.mult

    sb = ctx.enter_context(tc.tile_pool(name="persist", bufs=1))
    wpool = ctx.enter_context(tc.tile_pool(name="wpool", bufs=1))
    wns_pool = ctx.enter_context(tc.tile_pool(name="wns", bufs=2))
    stage = ctx.enter_context(tc.tile_pool(name="stage", bufs=3))
    accp = ctx.enter_context(tc.tile_pool(name="accp", bufs=2, space="PSUM"))
    dbgp = ctx.enter_context(tc.tile_pool(name="dbgp", bufs=1)) if DEBUG else None
    dram = ctx.enter_context(tc.tile_pool(name="dram", bufs=2, space="DRAM"))

    # ---- persistent SBUF tensors ----
    xT = sb.tile([128, 8 * 1024], BF)      # [d-part, dt*1024 + l]
    qT = sb.tile([128, 8 * 1024], BF)      # [e-part, et*1024 + l]
    kT = sb.tile([128, 8 * 1024], BF)
    vb = sb.tile([128, 8 * 1040], BF)      # [l-part, lt*1040 + h*65 + eh]; col h*65+64 = ones
    oT = sb.tile([128, 8 * 1024], BF)      # [e-part, et*1024 + l]
    xtails = sb.tile([128, 64], BF)        # [d-part, dt*8 + bb]
    sel = sb.tile([64, 8], BF)             # one-hot row selector (per-core data)
    mbias = sb.tile([128, 8], F32)         # key-padding additive bias per k-block
    tri4 = sb.tile([128, 512], BF)         # 4x repeated [128,128] lower-tri 0/1
    ones1 = sb.tile([1, 64], BF)
    nsb = sb.tile([8, 3072], BF)           # my position's q|k|v for all batches
    fullg = sb.tile([64, 3072], BF)        # gathered: row n*8+bb
    # per-head exp'd score tiles (bufs=1; subtile deps pipeline across pairs)
    pt0 = [sb.tile([128, 2560], BF, name=f"pt0_{i}") for i in range(2)]  # g=0
    pt1 = [sb.tile([128, 4608], BF, name=f"pt1_{i}") for i in range(2)]  # g=1

    wq = sb.tile([128, 8 * 1024], BF)
    wk = sb.tile([128, 8 * 1024], BF)
    wv = wpool.tile([128, 8 * 1024], BF, name="wslot")

    # ---- input DMAs (batched 3D APs, spread across engine queues) ----
    def load_w(eng, dst, name):
        d = dst[:].rearrange("p (t l) -> p t l", l=1024)
        s = aps[name][:, :].rearrange("(t p) l -> p t l", p=128)
        eng.dma_start(d, s)

    load_w(nc.sync, xT, "xT")
    load_w(nc.sync, wv, "wvT")
    load_w(nc.scalar, wq, "wqT")
    load_w(nc.scalar, wk, "wkT")
    nc.gpsimd.dma_start(
        xtails[:].rearrange("p (t b) -> p t b", b=8),
        aps["xtails"][:, :].rearrange("(t p) b -> p t b", p=128),
    )
    nc.gpsimd.dma_start(sel[:], aps["sel"][:])
    nc.sync.dma_start(mbias[:], aps["maskbias"][:])
    nc.sync.dma_start(tri4[:], aps["tri4"][:])
    nc.sync.dma_start(ones1[:], aps["onesb"][:])

    for t in pt0 + pt1:
        nc.vector.memset(t[:], 0.0)

    # ones columns of vb (col h*65+64 per lt): one strided memset
    nc.vector.memset(
        vb[:].rearrange("p (t h c) -> p t h c", h=16, c=65)[:, :, :, 64:65], 1.0
    )

    # ---- phase A: V projection, natural [l, e] layout interleaved per head ----
    def v_proj(eg):
        for lt in range(8):
            acc = accp.tile([128, 512], F32, name="acc")
            for dt in range(8):
                nc.tensor.matmul(
                    acc[:],
                    xT[:, dt * 1024 + lt * 128: dt * 1024 + lt * 128 + 128],
                    wv[:, dt * 1024 + eg * 512: dt * 1024 + eg * 512 + 512],
                    start=(dt == 0),
                    stop=(dt == 7),
                )
            base = lt * 1040 + eg * 520
            dst = vb[:, base: base + 520].rearrange("p (h c) -> p h c", c=65)[:, :, 0:64]
            src = acc[:].rearrange("p (h c) -> p h c", c=64)
            nc.vector.tensor_copy(dst, src)

    # ---- phase B: ns projections for my position (all batches) ----
    with tc.tile_pool(name="nsps", bufs=1, space="PSUM") as nsps:
        nspp = [nsps.tile([8, 512], F32, name=f"nsp{ck}") for ck in range(6)]
        for dt in range(8):
            wt = wns_pool.tile([128, 3072], BF)
            nc.gpsimd.dma_start(wt[:], aps["wnsT"][dt * 128:dt * 128 + 128, :])
            for ck in range(6):
                nc.tensor.matmul(
                    nspp[ck][:],
                    xtails[:, bass.ts(dt, 8)],
                    wt[:, bass.ts(ck, 512)],
                    start=(dt == 0),
                    stop=(dt == 7),
                )
        for ck in range(6):
            nc.vector.tensor_copy(nsb[:, bass.ts(ck, 512)], nspp[ck][:])

    gin = dram.tile([8, 3072], BF)
    gout = dram.tile([64, 3072], BF)
    nc.gpsimd.dma_start(gin[:], nsb[:])
    nc.gpsimd.collective_compute(
        "AllGather",
        mybir.AluOpType.bypass,
        replica_groups=[list(range(NCORES))],
        ins=[gin.opt()],
        outs=[gout.opt()],
    )
    nc.gpsimd.dma_start(fullg[:], gout[:])

    v_proj(0)
    v_proj(1)

    # wo reuses wv's buffer (pool bufs=3); DMA starts once V is done with wv
    wo = wpool.tile([128, 8 * 1024], BF, name="wslot")
    load_w(nc.gpsimd, wo, "woutT")

    spp = ctx.enter_context(tc.tile_pool(name="spp", bufs=4, space="PSUM"))
    opp = ctx.enter_context(tc.tile_pool(name="opp", bufs=2, space="PSUM"))

    def qk_proj(et):
        for w, dst in ((wq, qT), (wk, kT)):
            for lg in range(2):
                acc = accp.tile([128, 512], F32, name="acc")
                for dt in range(8):
                    nc.tensor.matmul(
                        acc[:],
                        w[:, dt * 1024 + et * 128: dt * 1024 + et * 128 + 128],
                        xT[:, dt * 1024 + lg * 512: dt * 1024 + lg * 512 + 512],
                        start=(dt == 0),
                        stop=(dt == 7),
                    )
                nc.vector.tensor_copy(
                    dst[:, et * 1024 + lg * 512: et * 1024 + lg * 512 + 512], acc[:]
                )

    def attn(et, g):
        """Scores (row-tiled 2-up) + exp + mask + AV + normalize for pair et."""
        nj = 4 * g + 4
        pts = pt0 if g == 0 else pt1
        qs = et * 1024 + g * 512
        # scores + exp, both heads interleaved per j (concurrent row tiles)
        sps = {}
        for j in range(nj):
            for half in range(2):
                r0 = half * 64
                sp = spp.tile([128, 512], F32, name="sp")
                nc.tensor.matmul(
                    sp[:],
                    kT[r0:r0 + 64, et * 1024 + j * 128: et * 1024 + j * 128 + 128],
                    qT[r0:r0 + 64, qs: qs + 512],
                    start=True,
                    stop=True,
                )
                sps[(j, half)] = sp
        for j in range(nj):
            lead = max(0, j * 128 - g * 512)
            for half in range(2):
                nc.scalar.activation(
                    pts[half][:, j * 512 + lead: j * 512 + 512],
                    sps[(j, half)][:, lead:512],
                    Exp,
                    bias=mbias[:, j: j + 1],
                    scale=0.125,
                )
        # causal mask: zero strictly-upper part of the 4 diagonal 128-blocks
        # (windows at moff + 640*w, batched in one strided op per head)
        moff = 2048 * g
        t3 = tri4[:].rearrange("p (w c) -> p w c", c=128)
        for half in range(2):
            w3 = (
                pts[half][:, moff: moff + 2560]
                .rearrange("p (w c) -> p w c", c=640)[:, :, 0:128]
            )
            nc.vector.tensor_tensor(w3, w3, t3, MUL)
        # AV (ones column in vb row 64 gives the softmax denominator)
        for half in range(2):
            h = 2 * et + half
            op = opp.tile([128, 512], F32, name="op")
            for j in range(nj):
                nc.tensor.matmul(
                    op[0:65, :],
                    vb[:, j * 1040 + h * 65: j * 1040 + h * 65 + 65],
                    pts[half][:, j * 512: j * 512 + 512],
                    start=(j == 0),
                    stop=(j == nj - 1),
                )
            den = stage.tile([1, 512], F32, name="den", bufs=2)
            nc.vector.tensor_copy(den[:], op[64:65, :])
            rec = stage.tile([1, 512], F32, name="rec", bufs=2)
            nc.vector.reciprocal_approx_fast(rec[:], den[:])
            rec16 = stage.tile([1, 512], BF, name="rec16", bufs=2)
            nc.vector.tensor_copy(rec16[:], rec[:])
            bp = spp.tile([128, 512], F32, name="sp")
            nc.tensor.matmul(
                bp[0:64, :],
                ones1[:],
                rec16[:],
                start=True,
                stop=True,
            )
            bb = stage.tile([64, 512], BF, name="bb")
            nc.vector.tensor_copy(bb[:], bp[0:64, :])
            if DEBUG and et == 0 and g == 0 and half == 0:
                dop = dbgp.tile([65, 512], F32, name="dop")
                nc.vector.tensor_copy(dop[:], op[0:65, :])
                nc.gpsimd.dma_start(aps["dbg_op"][:, :], dop[:])
                nc.gpsimd.dma_start(aps["dbg_rec"][:, :], rec[:])
                nc.gpsimd.dma_start(aps["dbg_bb"][:, :], bb[:])
            nc.vector.tensor_tensor(
                oT[half * 64: half * 64 + 64, qs: qs + 512],
                op[0:64, :],
                bb[:],
                MUL,
            )

    def out_proj(lt, eg):
        acc = accp.tile([128, 512], F32, name="acc")
        for et in range(8):
            nc.tensor.matmul(
                acc[:],
                oT[:, et * 1024 + lt * 128: et * 1024 + lt * 128 + 128],
                wo[:, et * 1024 + eg * 512: et * 1024 + eg * 512 + 512],
                start=(et == 0),
                stop=(et == 7),
            )
        ys = stage.tile([128, 512], F32, name="ys")
        nc.vector.tensor_copy(ys[:], acc[:])
        nc.gpsimd.dma_start(
            aps["y"][lt * 128: lt * 128 + 128, eg * 512: eg * 512 + 512], ys[:]
        )

    def tailmerge(et):
        for c2 in (et, 8 + et):
            tp = spp.tile([128, 512], F32, name="sp")
            nc.tensor.matmul(
                tp[:, 0:8],
                fullg[:, c2 * 128: c2 * 128 + 128],
                sel[:],
                start=True,
                stop=True,
            )
            dst = qT if c2 < 8 else kT
            nc.vector.tensor_copy(
                dst[:, et * 1024 + 1016: et * 1024 + 1024], tp[:, 0:8]
            )

    def vb_tails():
        for vg in range(2):
            tp = spp.tile([128, 512], F32, name="sp")
            nc.tensor.matmul(
                tp[0:8, :],
                sel[:],
                fullg[:, 2048 + vg * 512: 2048 + vg * 512 + 512],
                start=True,
                stop=True,
            )
            vt = stage.tile([8, 512], BF, name="vt", bufs=1)
            nc.vector.tensor_copy(vt[:], tp[0:8, :])
            dst = (
                vb[120:128, 7 * 1040 + vg * 520: 7 * 1040 + vg * 520 + 520]
                .rearrange("p (h c) -> p h c", c=65)[:, :, 0:64]
            )
            nc.gpsimd.dma_start(dst, vt[:].rearrange("p (h c) -> p h c", c=64))

    # ---- unified pipeline: QK(i) | g0+tails(i-1) | g1(i-2) ----
    for i in range(10):
        if i < 8:
            qk_proj(i)
        if 1 <= i <= 8:
            attn(i - 1, 0)
            tailmerge(i - 1)
            if i == 1:
                vb_tails()
        if 2 <= i <= 9:
            attn(i - 2, 1)

    # ---- output projection (all oT columns complete) ----
    for lt in range(8):
        for eg in range(2):
            out_proj(lt, eg)

    if DEBUG:
        nc.gpsimd.dma_start(aps["dbg_qT"][:, :], qT[:])
        nc.gpsimd.dma_start(aps["dbg_kT"][:, :], kT[:])
        nc.gpsimd.dma_start(aps["dbg_vb"][:, :], vb[:])
        nc.gpsimd.dma_start(aps["dbg_oT"][:, :], oT[:])
        nc.gpsimd.dma_start(aps["dbg_pt0"][:, :], pt0[0][:])


def _build():
    if "nc" in _CACHE:
        return _CACHE["nc"]
    nc = bacc.Bacc("TRN2", target_bir_lowering=False, debug=False, num_devices=NCORES)
    aps = {}
    for name, shape, dt in [
        ("xT", [1024, 1024], BF),
        ("wqT", [1024, 1024], BF),
        ("wkT", [1024, 1024], BF),
        ("wvT", [1024, 1024], BF),
        ("woutT", [1024, 1024], BF),
        ("wnsT", [1024, 3072], BF),
        ("xtails", [1024, 8], BF),
        ("sel", [64, 8], BF),
        ("maskbias", [128, 8], F32),
        ("tri4", [128, 512], BF),
        ("onesb", [1, 64], BF),
    ]:
        aps[name] = nc.dram_tensor(name, shape, dt, kind="ExternalInput").ap()
    aps["y"] = nc.dram_tensor("y", [1024, 1024], F32, kind="ExternalOutput").ap()
    if DEBUG:
        for name, shape, dt in [
            ("dbg_op", [65, 512], F32),
            ("dbg_rec", [1, 512], F32),
            ("dbg_bb", [64, 512], BF),
            ("dbg_qT", [128, 8192], BF),
            ("dbg_kT", [128, 8192], BF),
            ("dbg_vb", [128, 8320], BF),
            ("dbg_oT", [128, 8192], BF),
            ("dbg_pt0", [128, 2048], BF),
        ]:
            aps[name] = nc.dram_tensor(name, shape, dt, kind="ExternalOutput").ap()

    with tile.TileContext(nc) as tc:
        _attention_kernel(tc, aps)
    nc.compile()
    _CACHE["nc"] = nc
    return nc


def kernel(x, key_padding_mask, Wq_s, Wk_s, Wv_s, Wq_ns, Wk_ns, Wv_ns, W_out, **kw):
    x = np.asarray(x, np.float32)
    mask = np.asarray(key_padding_mask)
    bf = ml_dtypes.bfloat16

    wqT = np.ascontiguousarray(np.asarray(Wq_s, np.float32).T.astype(bf))
    wkT = np.ascontiguousarray(np.asarray(Wk_s, np.float32).T.astype(bf))
    wvT = np.ascontiguousarray(np.asarray(Wv_s, np.float32).T.astype(bf))
    woT = np.ascontiguousarray(np.asarray(W_out, np.float32).T.astype(bf))
    onesb = np.ones((1, 64), bf)

    # tri4: 4 copies of the [128,128] lower-triangular keep-mask (p <= c)
    p = np.arange(128)[:, None]
    c_ = np.arange(128)[None, :]
    tri01 = (p <= c_).astype(bf)
    tri4 = np.ascontiguousarray(np.tile(tri01, (1, 4)))

    Wq_ns = np.asarray(Wq_ns, np.float32)
    Wk_ns = np.asarray(Wk_ns, np.float32)
    Wv_ns = np.asarray(Wv_ns, np.float32)

    in_maps = []
    for c in range(NCORES):
        xT = np.ascontiguousarray(x[c].T.astype(bf))
        xtails = np.ascontiguousarray(x[:, LS + c, :].T.astype(bf))  # [1024 d, 8 bb]
        wnsT = np.ascontiguousarray(
            np.concatenate([Wq_ns[c].T, Wk_ns[c].T, Wv_ns[c].T], axis=1).astype(bf)
        )
        selm = np.zeros((64, 8), bf)
        for n in range(NS):
            selm[n * 8 + c, n] = 1.0
        mb = np.where(mask[c], 0.0, NEG).astype(np.float32).reshape(8, 128).T
        mb = np.ascontiguousarray(mb)
        in_maps.append(
            {
                "xT": xT,
                "wqT": wqT,
                "wkT": wkT,
                "wvT": wvT,
                "woutT": woT,
                "wnsT": wnsT,
                "xtails": xtails,
                "sel": selm,
                "maskbias": mb,
                "tri4": tri4,
                "onesb": onesb,
            }
        )

    nc = _build()
    res = run_bass_kernel_spmd(nc, in_maps, list(range(NCORES)), trace=TRACE)
    _CACHE["exec_time_ns"] = res.exec_time_ns
    _CACHE["res"] = res
    out = np.stack([res.results[c]["y"] for c in range(NCORES)], axis=0)
    return out.astype(np.float32)


# revision 14
# speedup vs baseline: 1.4112x; 1.0955x over previous
"""Mixed causal attention (B=8,L=1024,D=1024,H=16,NS=8) on 8 TRN2 cores.

Sharding: data-parallel over batch (core b owns batch b).  The per-position
(ns) projection weights are sharded by position: core c computes q/k/v for
position 1016+c for ALL batches, an AllGather distributes the results, and
each core extracts its batch's 8 tail rows with a one-hot selection matmul.

Pipeline (engine-balanced, keeps PE dense so HAM stays at full clock):
  A: V projection (psum-accumulated, strided interleave copy into vb)
  B: ns projections (6 psum banks accumulate over dt) -> AllGather (async)
  C: per head-pair i: QK proj strip et=i overlapped with g0 attention of
     pair i-1 (scores row-tiled 2-up, full-width exps, batched causal-mask
     multiply, fast reciprocal, fp32r broadcast matmul)
  T: tail merge (q/k/v rows 1016..1023 from gathered ns results)
  D: per pair i: g1 scores of pair i overlapped with AV of pair i-1 and
     one out-projection chunk (lt<4 columns complete after C)
  E: remaining out-projection chunks
"""

import sys
from contextlib import ExitStack

import numpy as np

sys.path.insert(0, "/opt/trn_rl_repo")

import ml_dtypes  # noqa: E402
import concourse.bass as bass  # noqa: E402
import concourse.tile as tile  # noqa: E402
from concourse import bacc, mybir  # noqa: E402
from concourse._compat import with_exitstack  # noqa: E402
from concourse.bass_utils import run_bass_kernel_spmd  # noqa: E402

B, L, D, H, NS = 8, 1024, 1024, 16, 8
HD = D // H          # 64
LS = L - NS          # 1016
NCORES = 8
NEG = -1.0e9
BF = mybir.dt.bfloat16
F32 = mybir.dt.float32
F32R = mybir.dt.float32r

_CACHE = {}
TRACE = False
DEBUG = False


@with_exitstack
def _attention_kernel(ctx: ExitStack, tc: tile.TileContext, aps: dict):
    nc = tc.nc
    Exp = mybir.ActivationFunctionType.Exp
    MUL = mybir.AluOpType.mult

    sb = ctx.enter_context(tc.tile_pool(name="persist", bufs=1))
    wpool = ctx.enter_context(tc.tile_pool(name="wpool", bufs=1))
    wns_pool = ctx.enter_context(tc.tile_pool(name="wns", bufs=2))
    stage = ctx.enter_context(tc.tile_pool(name="stage", bufs=3))
    accp = ctx.enter_context(tc.tile_pool(name="accp", bufs=2, space="PSUM"))
    dbgp = ctx.enter_context(tc.tile_pool(name="dbgp", bufs=1)) if DEBUG else None
    dram = ctx.enter_context(tc.tile_pool(name="dram", bufs=2, space="DRAM"))

    # ---- persistent SBUF tensors ----
    xT = sb.tile([128, 8 * 1024], BF)      # [d-part, dt*1024 + l]
    qT = sb.tile([128, 8 * 1024], BF)      # [e-part, et*1024 + l]
    kT = sb.tile([128, 8 * 1024], BF)
    vb = sb.tile([128, 8 * 1040], BF)      # [l-part, lt*1040 + h*65 + eh]; col h*65+64 = ones
    oT = sb.tile([128, 8 * 1024], BF)      # [e-part, et*1024 + l]
    xtails = sb.tile([128, 64], BF)        # [d-part, dt*8 + bb]
    sel = sb.tile([64, 8], BF)             # one-hot row selector (per-core data)
    mbias = sb.tile([128, 8], F32)         # key-padding additive bias per k-block
    tri4 = sb.tile([128, 512], BF)         # 4x repeated [128,128] lower-tri 0/1
    sel2 = sb.tile([33, 128], BF)
    den2 = sb.tile([33, 512], F32)
    rec2 = sb.tile([33, 512], F32)
    rec16 = sb.tile([33, 512], BF)
    nsb = sb.tile([8, 3072], BF)           # my position's q|k|v for all batches
    fullg = sb.tile([64, 3072], BF)        # gathered: row n*8+bb
    # per-head exp'd score tiles (bufs=1; subtile deps pipeline across pairs)
    pt0 = [sb.tile([128, 2560], BF, name=f"pt0_{i}") for i in range(2)]  # g=0
    pt1 = [sb.tile([128, 4608], BF, name=f"pt1_{i}") for i in range(2)]  # g=1

    wq = sb.tile([128, 8 * 1024], BF)
    wk = sb.tile([128, 8 * 1024], BF)
    wv = wpool.tile([128, 8 * 1024], BF, name="wslot")

    # ---- input DMAs (batched 3D APs, spread across engine queues) ----
    def load_w(eng, dst, name):
        d = dst[:].rearrange("p (t l) -> p t l", l=1024)
        s = aps[name][:, :].rearrange("(t p) l -> p t l", p=128)
        eng.dma_start(d, s)

    load_w(nc.sync, xT, "xT")
    load_w(nc.scalar, wv, "wvT")
    nc.gpsimd.dma_start(
        xtails[:].rearrange("p (t b) -> p t b", b=8),
        aps["xtails"][:, :].rearrange("(t p) b -> p t b", p=128),
    )
    nc.gpsimd.dma_start(sel[:], aps["sel"][:])
    nc.sync.dma_start(mbias[:], aps["maskbias"][:])
    nc.sync.dma_start(tri4[:], aps["tri4"][:])
    nc.sync.dma_start(sel2[:], aps["sel2"][:])
    load_w(nc.sync, wq, "wqT")
    load_w(nc.scalar, wk, "wkT")

    for t in pt0 + pt1:
        nc.gpsimd.memset(t[:], 0.0)
    nc.gpsimd.memset(den2[:], 1.0)

    # ones columns of vb (col h*65+64 per lt): one strided memset
    nc.vector.memset(
        vb[:].rearrange("p (t h c) -> p t h c", h=16, c=65)[:, :, :, 64:65], 1.0
    )

    # ---- phase A: V projection, natural [l, e] layout interleaved per head ----
    def v_proj(eg):
        for lt in range(8):
            acc = accp.tile([128, 512], F32, name="acc")
            for dt in range(8):
                nc.tensor.matmul(
                    acc[:],
                    xT[:, dt * 1024 + lt * 128: dt * 1024 + lt * 128 + 128],
                    wv[:, dt * 1024 + eg * 512: dt * 1024 + eg * 512 + 512],
                    start=(dt == 0),
                    stop=(dt == 7),
                )
            base = lt * 1040 + eg * 520
            dst = vb[:, base: base + 520].rearrange("p (h c) -> p h c", c=65)[:, :, 0:64]
            src = acc[:].rearrange("p (h c) -> p h c", c=64)
            nc.vector.tensor_copy(dst, src)

    # ---- phase B: ns projections for my position (all batches) ----
    with tc.tile_pool(name="nsps", bufs=1, space="PSUM") as nsps:
        nspp = [nsps.tile([8, 512], F32, name=f"nsp{ck}") for ck in range(6)]
        for dt in range(8):
            wt = wns_pool.tile([128, 3072], BF)
            nc.gpsimd.dma_start(wt[:], aps["wnsT"][dt * 128:dt * 128 + 128, :])
            for ck in range(6):
                nc.tensor.matmul(
                    nspp[ck][:],
                    xtails[:, bass.ts(dt, 8)],
                    wt[:, bass.ts(ck, 512)],
                    start=(dt == 0),
                    stop=(dt == 7),
                )
        for ck in range(6):
            nc.vector.tensor_copy(nsb[:, bass.ts(ck, 512)], nspp[ck][:])

    gin = dram.tile([8, 3072], BF)
    gout = dram.tile([64, 3072], BF)
    nc.gpsimd.dma_start(gin[:], nsb[:])
    nc.gpsimd.collective_compute(
        "AllGather",
        mybir.AluOpType.bypass,
        replica_groups=[list(range(NCORES))],
        ins=[gin.opt()],
        outs=[gout.opt()],
    )
    nc.gpsimd.dma_start(fullg[:], gout[:])

    v_proj(0)
    v_proj(1)

    # wo reuses wv's buffer (pool bufs=3); DMA starts once V is done with wv
    wo = wpool.tile([128, 8 * 1024], BF, name="wslot")
    load_w(nc.gpsimd, wo, "woutT")

    spp = ctx.enter_context(tc.tile_pool(name="spp", bufs=4, space="PSUM"))
    opp = ctx.enter_context(tc.tile_pool(name="opp", bufs=2, space="PSUM"))

    def qk_proj(et):
        for w, dst in ((wq, qT), (wk, kT)):
            for lg in range(2):
                acc = accp.tile([128, 512], F32, name="acc")
                for dt in range(8):
                    nc.tensor.matmul(
                        acc[:],
                        w[:, dt * 1024 + et * 128: dt * 1024 + et * 128 + 128],
                        xT[:, dt * 1024 + lg * 512: dt * 1024 + lg * 512 + 512],
                        start=(dt == 0),
                        stop=(dt == 7),
                    )
                nc.vector.tensor_copy(
                    dst[:, et * 1024 + lg * 512: et * 1024 + lg * 512 + 512], acc[:]
                )

    def attn(et, g):
        """Scores (row-tiled 2-up) + exp + mask + AV + normalize for pair et."""
        nj = 4 * g + 4
        pts = pt0 if g == 0 else pt1
        qs = et * 1024 + g * 512
        # scores + exp, both heads interleaved per j (concurrent row tiles)
        sps = {}
        for j in range(nj):
            lead = max(0, j * 128 - g * 512)
            for half in range(2):
                r0 = half * 64
                sp = spp.tile([128, 512], F32, name="sp")
                nc.tensor.matmul(
                    sp[:, lead:512],
                    kT[r0:r0 + 64, et * 1024 + j * 128: et * 1024 + j * 128 + 128],
                    qT[r0:r0 + 64, qs + lead: qs + 512],
                    start=True,
                    stop=True,
                )
                sps[(j, half)] = sp
        for j in range(nj):
            lead = max(0, j * 128 - g * 512)
            for half in range(2):
                nc.scalar.activation(
                    pts[half][:, j * 512 + lead: j * 512 + 512],
                    sps[(j, half)][:, lead:512],
                    Exp,
                    bias=mbias[:, j: j + 1],
                    scale=0.125,
                )
        # causal mask: zero strictly-upper part of the 4 diagonal 128-blocks
        # (windows at moff + 640*w, batched in one strided op per head)
        moff = 2048 * g
        t3 = tri4[:].rearrange("p (w c) -> p w c", c=128)
        for half in range(2):
            w3 = (
                pts[half][:, moff: moff + 2560]
                .rearrange("p (w c) -> p w c", c=640)[:, :, 0:128]
            )
            nc.vector.tensor_tensor(w3, w3, t3, MUL)
        # AV (ones column in vb row 64 gives the softmax denominator)
        ops = []
        for half in range(2):
            h = 2 * et + half
            op = opp.tile([128, 512], F32, name="op")
            for j in range(nj):
                lead = max(0, j * 128 - g * 512)
                nc.tensor.matmul(
                    op[0:65, lead:512],
                    vb[:, j * 1040 + h * 65: j * 1040 + h * 65 + 65],
                    pts[half][:, j * 512 + lead: j * 512 + 512],
                    start=(j == 0),
                    stop=(j == nj - 1),
                )
            ops.append(op)
        # fused normalization for both heads: one recip / broadcast / cast
        nc.vector.tensor_copy(den2[0:1, :], ops[0][64:65, :])
        nc.vector.tensor_copy(den2[32:33, :], ops[1][64:65, :])
        nc.vector.reciprocal_approx_fast(rec2[:], den2[:])
        nc.vector.tensor_copy(rec16[:], rec2[:])
        bp = spp.tile([128, 512], F32, name="sp")
        nc.tensor.matmul(bp[:], sel2[:], rec16[:], start=True, stop=True)
        bb = stage.tile([128, 512], BF, name="bb")
        nc.vector.tensor_copy(bb[:], bp[:])
        if DEBUG and et == 0 and g == 0:
            dop = dbgp.tile([65, 512], F32, name="dop")
            nc.vector.tensor_copy(dop[:], ops[0][0:65, :])
            nc.gpsimd.dma_start(aps["dbg_op"][:, :], dop[:])
            nc.gpsimd.dma_start(aps["dbg_rec"][:, :], rec2[0:1, :])
            nc.gpsimd.dma_start(aps["dbg_bb"][:, :], bb[0:64, :])
        for half in range(2):
            nc.vector.tensor_tensor(
                oT[half * 64: half * 64 + 64, qs: qs + 512],
                ops[half][0:64, :],
                bb[half * 64: half * 64 + 64, :],
                MUL,
            )

    def out_proj(lt, eg):
        acc = accp.tile([128, 512], F32, name="acc")
        for et in range(8):
            nc.tensor.matmul(
                acc[:],
                oT[:, et * 1024 + lt * 128: et * 1024 + lt * 128 + 128],
                wo[:, et * 1024 + eg * 512: et * 1024 + eg * 512 + 512],
                start=(et == 0),
                stop=(et == 7),
            )
        ys = stage.tile([128, 512], F32, name="ys")
        nc.vector.tensor_copy(ys[:], acc[:])
        nc.gpsimd.dma_start(
            aps["y"][lt * 128: lt * 128 + 128, eg * 512: eg * 512 + 512], ys[:]
        )

    def tailmerge(et):
        for c2 in (et, 8 + et):
            tp = spp.tile([128, 512], F32, name="sp")
            nc.tensor.matmul(
                tp[:, 0:8],
                fullg[:, c2 * 128: c2 * 128 + 128],
                sel[:],
                start=True,
                stop=True,
            )
            dst = qT if c2 < 8 else kT
            nc.vector.tensor_copy(
                dst[:, et * 1024 + 1016: et * 1024 + 1024], tp[:, 0:8]
            )

    def vb_tails():
        for vg in range(2):
            tp = spp.tile([128, 512], F32, name="sp")
            nc.tensor.matmul(
                tp[0:8, :],
                sel[:],
                fullg[:, 2048 + vg * 512: 2048 + vg * 512 + 512],
                start=True,
                stop=True,
            )
            vt = stage.tile([8, 512], BF, name="vt", bufs=1)
            nc.vector.tensor_copy(vt[:], tp[0:8, :])
            dst = (
                vb[120:128, 7 * 1040 + vg * 520: 7 * 1040 + vg * 520 + 520]
                .rearrange("p (h c) -> p h c", c=65)[:, :, 0:64]
            )
            nc.gpsimd.dma_start(dst, vt[:].rearrange("p (h c) -> p h c", c=64))

    # ---- unified pipeline: QK(i) | g0+tails(i-1) | g1(i-2) ----
    for i in range(10):
        if i < 8:
            qk_proj(i)
        if 1 <= i <= 8:
            attn(i - 1, 0)
            tailmerge(i - 1)
            if i == 1:
                vb_tails()
        if 2 <= i <= 9:
            attn(i - 2, 1)
        if i == 9:
            for lt in range(4):
                for eg in range(2):
                    out_proj(lt, eg)

    # ---- remaining output projection (needs all g1 done) ----
    for lt in range(4, 8):
        for eg in range(2):
            out_proj(lt, eg)

    if DEBUG:
        nc.gpsimd.dma_start(aps["dbg_qT"][:, :], qT[:])
        nc.gpsimd.dma_start(aps["dbg_kT"][:, :], kT[:])
        nc.gpsimd.dma_start(aps["dbg_vb"][:, :], vb[:])
        nc.gpsimd.dma_start(aps["dbg_oT"][:, :], oT[:])
        nc.gpsimd.dma_start(aps["dbg_pt0"][:, :], pt0[0][:])


def _build():
    if "nc" in _CACHE:
        return _CACHE["nc"]
    nc = bacc.Bacc("TRN2", target_bir_lowering=False, debug=False, num_devices=NCORES)
    aps = {}
    for name, shape, dt in [
        ("xT", [1024, 1024], BF),
        ("wqT", [1024, 1024], BF),
        ("wkT", [1024, 1024], BF),
        ("wvT", [1024, 1024], BF),
        ("woutT", [1024, 1024], BF),
        ("wnsT", [1024, 3072], BF),
        ("xtails", [1024, 8], BF),
        ("sel", [64, 8], BF),
        ("maskbias", [128, 8], F32),
        ("tri4", [128, 512], BF),
        ("sel2", [33, 128], BF),
    ]:
        aps[name] = nc.dram_tensor(name, shape, dt, kind="ExternalInput").ap()
    aps["y"] = nc.dram_tensor("y", [1024, 1024], F32, kind="ExternalOutput").ap()
    if DEBUG:
        for name, shape, dt in [
            ("dbg_op", [65, 512], F32),
            ("dbg_rec", [1, 512], F32),
            ("dbg_bb", [64, 512], BF),
            ("dbg_qT", [128, 8192], BF),
            ("dbg_kT", [128, 8192], BF),
            ("dbg_vb", [128, 8320], BF),
            ("dbg_oT", [128, 8192], BF),
            ("dbg_pt0", [128, 2048], BF),
        ]:
            aps[name] = nc.dram_tensor(name, shape, dt, kind="ExternalOutput").ap()

    with tile.TileContext(nc) as tc:
        _attention_kernel(tc, aps)
    nc.compile()
    _CACHE["nc"] = nc
    return nc


def kernel(x, key_padding_mask, Wq_s, Wk_s, Wv_s, Wq_ns, Wk_ns, Wv_ns, W_out, **kw):
    x = np.asarray(x, np.float32)
    mask = np.asarray(key_padding_mask)
    bf = ml_dtypes.bfloat16

    wqT = np.ascontiguousarray(np.asarray(Wq_s, np.float32).T.astype(bf))
    wkT = np.ascontiguousarray(np.asarray(Wk_s, np.float32).T.astype(bf))
    wvT = np.ascontiguousarray(np.asarray(Wv_s, np.float32).T.astype(bf))
    woT = np.ascontiguousarray(np.asarray(W_out, np.float32).T.astype(bf))
    sel2 = np.zeros((33, 128), bf)
    sel2[0, 0:64] = 1.0
    sel2[32, 64:128] = 1.0

    # tri4: 4 copies of the [128,128] lower-triangular keep-mask (p <= c)
    p = np.arange(128)[:, None]
    c_ = np.arange(128)[None, :]
    tri01 = (p <= c_).astype(bf)
    tri4 = np.ascontiguousarray(np.tile(tri01, (1, 4)))

    Wq_ns = np.asarray(Wq_ns, np.float32)
    Wk_ns = np.asarray(Wk_ns, np.float32)
    Wv_ns = np.asarray(Wv_ns, np.float32)

    in_maps = []
    for c in range(NCORES):
        xT = np.ascontiguousarray(x[c].T.astype(bf))
        xtails = np.ascontiguousarray(x[:, LS + c, :].T.astype(bf))  # [1024 d, 8 bb]
        wnsT = np.ascontiguousarray(
            np.concatenate([Wq_ns[c].T, Wk_ns[c].T, Wv_ns[c].T], axis=1).astype(bf)
        )
        selm = np.zeros((64, 8), bf)
        for n in range(NS):
            selm[n * 8 + c, n] = 1.0
        mb = np.where(mask[c], 0.0, NEG).astype(np.float32).reshape(8, 128).T
        mb = np.ascontiguousarray(mb)
        in_maps.append(
            {
                "xT": xT,
                "wqT": wqT,
                "wkT": wkT,
                "wvT": wvT,
                "woutT": woT,
                "wnsT": wnsT,
                "xtails": xtails,
                "sel": selm,
                "maskbias": mb,
                "tri4": tri4,
                "sel2": sel2,
            }
        )

    nc = _build()
    res = run_bass_kernel_spmd(nc, in_maps, list(range(NCORES)), trace=TRACE)
    _CACHE["exec_time_ns"] = res.exec_time_ns
    _CACHE["res"] = res
    out = np.stack([res.results[c]["y"] for c in range(NCORES)], axis=0)
    return out.astype(np.float32)


# revision 16
# speedup vs baseline: 1.6125x; 1.1427x over previous
"""Mixed causal attention (B=8,L=1024,D=1024,H=16,NS=8) on 8 TRN2 cores.

Sharding: data-parallel over batch (core b owns batch b).  The per-position
(ns) projection weights are sharded by position: core c computes q/k/v for
position 1016+c for ALL batches, an AllGather distributes the results, and
each core extracts its batch's 8 tail rows with a one-hot selection matmul.

Pipeline (engine-balanced, keeps PE dense so HAM stays at full clock):
  A: V projection (psum-accumulated, strided interleave copy into vb)
  B: ns projections (6 psum banks accumulate over dt) -> AllGather (async)
  C: per head-pair i: QK proj strip et=i overlapped with g0 attention of
     pair i-1 (scores row-tiled 2-up, full-width exps, batched causal-mask
     multiply, fast reciprocal, fp32r broadcast matmul)
  T: tail merge (q/k/v rows 1016..1023 from gathered ns results)
  D: per pair i: g1 scores of pair i overlapped with AV of pair i-1 and
     one out-projection chunk (lt<4 columns complete after C)
  E: remaining out-projection chunks
"""

import sys
from contextlib import ExitStack

import numpy as np

sys.path.insert(0, "/opt/trn_rl_repo")

import ml_dtypes  # noqa: E402
import concourse.bass as bass  # noqa: E402
import concourse.tile as tile  # noqa: E402
from concourse import bacc, mybir  # noqa: E402
from concourse._compat import with_exitstack  # noqa: E402
from concourse.bass_utils import run_bass_kernel_spmd  # noqa: E402

B, L, D, H, NS = 8, 1024, 1024, 16, 8
HD = D // H          # 64
LS = L - NS          # 1016
NCORES = 8
NEG = -1.0e9
BF = mybir.dt.bfloat16
F32 = mybir.dt.float32
F32R = mybir.dt.float32r

_CACHE = {}
TRACE = False
DEBUG = False


@with_exitstack
def _attention_kernel(ctx: ExitStack, tc: tile.TileContext, aps: dict):
    nc = tc.nc
    Exp = mybir.ActivationFunctionType.Exp
    MUL = mybir.AluOpType.mult

    sb = ctx.enter_context(tc.tile_pool(name="persist", bufs=1))
    wpool = ctx.enter_context(tc.tile_pool(name="wpool", bufs=1))
    wns_pool = ctx.enter_context(tc.tile_pool(name="wns", bufs=2))
    stage = ctx.enter_context(tc.tile_pool(name="stage", bufs=3))
    accp = ctx.enter_context(tc.tile_pool(name="accp", bufs=2, space="PSUM"))
    dbgp = ctx.enter_context(tc.tile_pool(name="dbgp", bufs=1)) if DEBUG else None
    dram = ctx.enter_context(tc.tile_pool(name="dram", bufs=2, space="DRAM"))

    # ---- persistent SBUF tensors ----
    xT = sb.tile([128, 8 * 1024], BF)      # [d-part, dt*1024 + l]
    qT = sb.tile([128, 8 * 1024], BF)      # [e-part, et*1024 + l]
    kT = sb.tile([128, 8 * 1024], BF)
    vb = sb.tile([128, 8 * 1040], BF)      # [l-part, lt*1040 + h*65 + eh]; col h*65+64 = ones
    oT = sb.tile([128, 8 * 1024], BF)      # [e-part, et*1024 + l]
    xtails = sb.tile([128, 64], BF)        # [d-part, dt*8 + bb]
    sel = sb.tile([64, 8], BF)             # one-hot row selector (per-core data)
    mbias = sb.tile([128, 8], F32)         # key-padding additive bias per k-block
    tri4 = sb.tile([128, 512], BF)         # 4x repeated [128,128] lower-tri 0/1
    den2 = sb.tile([33, 512], F32)
    rec2 = sb.tile([33, 512], F32)
    nsb = sb.tile([8, 3072], BF)           # my position's q|k|v for all batches
    fullg = sb.tile([64, 3072], BF)        # gathered: row n*8+bb
    # per-head exp'd score tiles (bufs=1; subtile deps pipeline across pairs)
    pt0 = [sb.tile([128, 2560], BF, name=f"pt0_{i}") for i in range(2)]  # g=0
    pt1 = [sb.tile([128, 4608], BF, name=f"pt1_{i}") for i in range(2)]  # g=1

    wq = sb.tile([128, 8 * 1024], BF)
    wk = sb.tile([128, 8 * 1024], BF)
    wv = wpool.tile([128, 8 * 1024], BF, name="wslot")

    # ---- input DMAs (batched 3D APs, spread across engine queues) ----
    def load_w(eng, dst, name):
        d = dst[:].rearrange("p (t l) -> p t l", l=1024)
        s = aps[name][:, :].rearrange("(t p) l -> p t l", p=128)
        eng.dma_start(d, s)

    load_w(nc.sync, xT, "xT")
    load_w(nc.scalar, wv, "wvT")
    nc.gpsimd.dma_start(xtails[:], aps["xtails"][:, :])
    nc.gpsimd.dma_start(sel[:], aps["sel"][:])
    nc.sync.dma_start(mbias[:], aps["maskbias"][:])
    nc.sync.dma_start(tri4[:], aps["tri4"][:])
    load_w(nc.sync, wq, "wqT")
    load_w(nc.scalar, wk, "wkT")

    for t in pt0 + pt1:
        nc.gpsimd.memset(t[:], 0.0)
    nc.gpsimd.memset(den2[:], 1.0)

    # ones columns of vb (col h*65+64 per lt): one strided memset
    nc.vector.memset(
        vb[:].rearrange("p (t h c) -> p t h c", h=16, c=65)[:, :, :, 64:65], 1.0
    )

    # ---- phase A: V projection, natural [l, e] layout interleaved per head ----
    def v_proj(eg):
        for lt in range(8):
            acc = accp.tile([128, 512], F32, name="acc")
            for dt in range(8):
                nc.tensor.matmul(
                    acc[:],
                    xT[:, dt * 1024 + lt * 128: dt * 1024 + lt * 128 + 128],
                    wv[:, dt * 1024 + eg * 512: dt * 1024 + eg * 512 + 512],
                    start=(dt == 0),
                    stop=(dt == 7),
                )
            base = lt * 1040 + eg * 520
            dst = vb[:, base: base + 520].rearrange("p (h c) -> p h c", c=65)[:, :, 0:64]
            src = acc[:].rearrange("p (h c) -> p h c", c=64)
            nc.vector.tensor_copy(dst, src)

    # ---- phase B: ns projections for my position (all batches) ----
    with tc.tile_pool(name="nsps", bufs=1, space="PSUM") as nsps:
        nspp = [nsps.tile([8, 512], F32, name=f"nsp{ck}") for ck in range(6)]
        for dt in range(8):
            wt = wns_pool.tile([128, 3072], BF)
            nc.gpsimd.dma_start(wt[:], aps["wnsT"][dt * 128:dt * 128 + 128, :])
            for ck in range(6):
                nc.tensor.matmul(
                    nspp[ck][:],
                    xtails[:, bass.ts(dt, 8)],
                    wt[:, bass.ts(ck, 512)],
                    start=(dt == 0),
                    stop=(dt == 7),
                )
        for ck in range(6):
            nc.vector.tensor_copy(nsb[:, bass.ts(ck, 512)], nspp[ck][:])

    gin = dram.tile([8, 3072], BF)
    gout = dram.tile([64, 3072], BF)
    nc.gpsimd.dma_start(gin[:], nsb[:])
    nc.gpsimd.collective_compute(
        "AllGather",
        mybir.AluOpType.bypass,
        replica_groups=[list(range(NCORES))],
        ins=[gin.opt()],
        outs=[gout.opt()],
    )
    nc.gpsimd.dma_start(fullg[:], gout[:])

    v_proj(0)
    v_proj(1)

    # wo reuses wv's buffer (pool bufs=3); DMA starts once V is done with wv
    wo = wpool.tile([128, 8 * 1024], BF, name="wslot")
    load_w(nc.gpsimd, wo, "woutT")

    spp = ctx.enter_context(tc.tile_pool(name="spp", bufs=4, space="PSUM"))
    opp = ctx.enter_context(tc.tile_pool(name="opp", bufs=2, space="PSUM"))

    def qk_proj(et):
        for w, dst in ((wq, qT), (wk, kT)):
            for lg in range(2):
                acc = accp.tile([128, 512], F32, name="acc")
                for dt in range(8):
                    nc.tensor.matmul(
                        acc[:],
                        w[:, dt * 1024 + et * 128: dt * 1024 + et * 128 + 128],
                        xT[:, dt * 1024 + lg * 512: dt * 1024 + lg * 512 + 512],
                        start=(dt == 0),
                        stop=(dt == 7),
                    )
                nc.vector.tensor_copy(
                    dst[:, et * 1024 + lg * 512: et * 1024 + lg * 512 + 512], acc[:]
                )

    def attn(et, g):
        """Scores (row-tiled 2-up) + exp + mask + AV + normalize for pair et."""
        nj = 4 * g + 4
        pts = pt0 if g == 0 else pt1
        qs = et * 1024 + g * 512
        # scores + exp, both heads interleaved per j (concurrent row tiles)
        sps = {}
        for j in range(nj):
            lead = max(0, j * 128 - g * 512)
            for half in range(2):
                r0 = half * 64
                sp = spp.tile([128, 512], F32, name="sp")
                nc.tensor.matmul(
                    sp[:, lead:512],
                    kT[r0:r0 + 64, et * 1024 + j * 128: et * 1024 + j * 128 + 128],
                    qT[r0:r0 + 64, qs + lead: qs + 512],
                    start=True,
                    stop=True,
                )
                sps[(j, half)] = sp
        for j in range(nj):
            lead = max(0, j * 128 - g * 512)
            for half in range(2):
                nc.scalar.activation(
                    pts[half][:, j * 512 + lead: j * 512 + 512],
                    sps[(j, half)][:, lead:512],
                    Exp,
                    bias=mbias[:, j: j + 1],
                    scale=0.125,
                )
        # causal mask: zero strictly-upper part of the 4 diagonal 128-blocks
        # (windows at moff + 640*w, batched in one strided op per head)
        moff = 2048 * g
        t3 = tri4[:].rearrange("p (w c) -> p w c", c=128)
        for half in range(2):
            w3 = (
                pts[half][:, moff: moff + 2560]
                .rearrange("p (w c) -> p w c", c=640)[:, :, 0:128]
            )
            nc.vector.tensor_tensor(w3, w3, t3, MUL)
        # AV (ones column in vb row 64 gives the softmax denominator)
        ops = []
        for half in range(2):
            h = 2 * et + half
            op = opp.tile([128, 512], F32, name="op")
            for j in range(nj):
                lead = max(0, j * 128 - g * 512)
                nc.tensor.matmul(
                    op[0:65, lead:512],
                    vb[:, j * 1040 + h * 65: j * 1040 + h * 65 + 65],
                    pts[half][:, j * 512 + lead: j * 512 + 512],
                    start=(j == 0),
                    stop=(j == nj - 1),
                )
            ops.append(op)
        # fused normalization: one recip for both heads, then gpsimd
        # partition-broadcasts (base-0 in/out only) replace the bp matmul
        nc.vector.tensor_copy(den2[0:1, :], ops[0][64:65, :])
        nc.vector.tensor_copy(den2[32:33, :], ops[1][64:65, :])
        nc.vector.reciprocal_approx_fast(rec2[:], den2[:])
        bbs = []
        for half in range(2):
            r16 = stage.tile([1, 512], BF, name="r16", bufs=2)
            nc.vector.tensor_copy(r16[:], rec2[32 * half: 32 * half + 1, :])
            bb = stage.tile([64, 512], BF, name="bb", bufs=2)
            nc.gpsimd.partition_broadcast(bb[:], r16[:])
            bbs.append(bb)
        if DEBUG and et == 0 and g == 0:
            dop = dbgp.tile([65, 512], F32, name="dop")
            nc.vector.tensor_copy(dop[:], ops[0][0:65, :])
            nc.gpsimd.dma_start(aps["dbg_op"][:, :], dop[:])
            nc.gpsimd.dma_start(aps["dbg_rec"][:, :], rec2[0:1, :])
            nc.gpsimd.dma_start(aps["dbg_bb"][:, :], bbs[0][:])
        for half in range(2):
            nc.vector.tensor_tensor(
                oT[half * 64: half * 64 + 64, qs: qs + 512],
                ops[half][0:64, :],
                bbs[half][:],
                MUL,
            )

    def out_proj(lt, eg):
        acc = accp.tile([128, 512], F32, name="acc")
        for et in range(8):
            nc.tensor.matmul(
                acc[:],
                oT[:, et * 1024 + lt * 128: et * 1024 + lt * 128 + 128],
                wo[:, et * 1024 + eg * 512: et * 1024 + eg * 512 + 512],
                start=(et == 0),
                stop=(et == 7),
            )
        ys = stage.tile([128, 512], F32, name="ys")
        nc.vector.tensor_copy(ys[:], acc[:])
        nc.gpsimd.dma_start(
            aps["y"][lt * 128: lt * 128 + 128, eg * 512: eg * 512 + 512], ys[:]
        )

    def tailmerge(et):
        for c2 in (et, 8 + et):
            tp = spp.tile([128, 512], F32, name="sp")
            nc.tensor.matmul(
                tp[:, 0:8],
                fullg[:, c2 * 128: c2 * 128 + 128],
                sel[:],
                start=True,
                stop=True,
            )
            dst = qT if c2 < 8 else kT
            nc.vector.tensor_copy(
                dst[:, et * 1024 + 1016: et * 1024 + 1024], tp[:, 0:8]
            )

    def vb_tails():
        for vg in range(2):
            tp = spp.tile([128, 512], F32, name="sp")
            nc.tensor.matmul(
                tp[0:8, :],
                sel[:],
                fullg[:, 2048 + vg * 512: 2048 + vg * 512 + 512],
                start=True,
                stop=True,
            )
            vt = stage.tile([8, 512], BF, name="vt", bufs=1)
            nc.vector.tensor_copy(vt[:], tp[0:8, :])
            dst = (
                vb[120:128, 7 * 1040 + vg * 520: 7 * 1040 + vg * 520 + 520]
                .rearrange("p (h c) -> p h c", c=65)[:, :, 0:64]
            )
            nc.gpsimd.dma_start(dst, vt[:].rearrange("p (h c) -> p h c", c=64))

    # ---- unified pipeline: QK(i) | g0+tails(i-1) | g1(i-2) ----
    for i in range(10):
        if i < 8:
            qk_proj(i)
        if 1 <= i <= 8:
            attn(i - 1, 0)
            tailmerge(i - 1)
            if i == 1:
                vb_tails()
        if 2 <= i <= 9:
            attn(i - 2, 1)
        if i == 9:
            for lt in range(4):
                for eg in range(2):
                    out_proj(lt, eg)

    # ---- remaining output projection (needs all g1 done) ----
    for lt in range(4, 8):
        for eg in range(2):
            out_proj(lt, eg)

    if DEBUG:
        nc.gpsimd.dma_start(aps["dbg_qT"][:, :], qT[:])
        nc.gpsimd.dma_start(aps["dbg_kT"][:, :], kT[:])
        nc.gpsimd.dma_start(aps["dbg_vb"][:, :], vb[:])
        nc.gpsimd.dma_start(aps["dbg_oT"][:, :], oT[:])
        nc.gpsimd.dma_start(aps["dbg_pt0"][:, :], pt0[0][:, 0:2048])


def _build():
    if "nc" in _CACHE:
        return _CACHE["nc"]
    nc = bacc.Bacc("TRN2", target_bir_lowering=False, debug=False, num_devices=NCORES)
    aps = {}
    for name, shape, dt in [
        ("xT", [1024, 1024], BF),
        ("wqT", [1024, 1024], BF),
        ("wkT", [1024, 1024], BF),
        ("wvT", [1024, 1024], BF),
        ("woutT", [1024, 1024], BF),
        ("wnsT", [1024, 3072], BF),
        ("xtails", [128, 64], BF),
        ("sel", [64, 8], BF),
        ("maskbias", [128, 8], F32),
        ("tri4", [128, 512], BF),
    ]:
        aps[name] = nc.dram_tensor(name, shape, dt, kind="ExternalInput").ap()
    aps["y"] = nc.dram_tensor("y", [1024, 1024], F32, kind="ExternalOutput").ap()
    if DEBUG:
        for name, shape, dt in [
            ("dbg_op", [65, 512], F32),
            ("dbg_rec", [1, 512], F32),
            ("dbg_bb", [64, 512], BF),
            ("dbg_qT", [128, 8192], BF),
            ("dbg_kT", [128, 8192], BF),
            ("dbg_vb", [128, 8320], BF),
            ("dbg_oT", [128, 8192], BF),
            ("dbg_pt0", [128, 2048], BF),
        ]:
            aps[name] = nc.dram_tensor(name, shape, dt, kind="ExternalOutput").ap()

    with tile.TileContext(nc) as tc:
        _attention_kernel(tc, aps)
    nc.compile()
    _CACHE["nc"] = nc
    return nc


def kernel(x, key_padding_mask, Wq_s, Wk_s, Wv_s, Wq_ns, Wk_ns, Wv_ns, W_out, **kw):
    x = np.asarray(x, np.float32)
    mask = np.asarray(key_padding_mask)
    bf = ml_dtypes.bfloat16

    wqT = np.ascontiguousarray(np.asarray(Wq_s, np.float32).T.astype(bf))
    wkT = np.ascontiguousarray(np.asarray(Wk_s, np.float32).T.astype(bf))
    wvT = np.ascontiguousarray(np.asarray(Wv_s, np.float32).T.astype(bf))
    woT = np.ascontiguousarray(np.asarray(W_out, np.float32).T.astype(bf))

    # tri4: 4 copies of the [128,128] lower-triangular keep-mask (p <= c)
    p = np.arange(128)[:, None]
    c_ = np.arange(128)[None, :]
    tri01 = (p <= c_).astype(bf)
    tri4 = np.ascontiguousarray(np.tile(tri01, (1, 4)))

    Wq_ns = np.asarray(Wq_ns, np.float32)
    Wk_ns = np.asarray(Wk_ns, np.float32)
    Wv_ns = np.asarray(Wv_ns, np.float32)

    in_maps = []
    for c in range(NCORES):
        xT = np.ascontiguousarray(x[c].T.astype(bf))
        xt_full = x[:, LS + c, :].T.astype(bf)  # [1024 d, 8 bb]
        xtails = np.ascontiguousarray(
            xt_full.reshape(8, 128, 8).transpose(1, 0, 2).reshape(128, 64)
        )
        wnsT = np.ascontiguousarray(
            np.concatenate([Wq_ns[c].T, Wk_ns[c].T, Wv_ns[c].T], axis=1).astype(bf)
        )
        selm = np.zeros((64, 8), bf)
        for n in range(NS):
            selm[n * 8 + c, n] = 1.0
        mb = np.where(mask[c], 0.0, NEG).astype(np.float32).reshape(8, 128).T
        mb = np.ascontiguousarray(mb)
        in_maps.append(
            {
                "xT": xT,
                "wqT": wqT,
                "wkT": wkT,
                "wvT": wvT,
                "woutT": woT,
                "wnsT": wnsT,
                "xtails": xtails,
                "sel": selm,
                "maskbias": mb,
                "tri4": tri4,
            }
        )

    nc = _build()
    res = run_bass_kernel_spmd(nc, in_maps, list(range(NCORES)), trace=TRACE)
    _CACHE["exec_time_ns"] = res.exec_time_ns
    _CACHE["res"] = res
    out = np.stack([res.results[c]["y"] for c in range(NCORES)], axis=0)
    return out.astype(np.float32)


# revision 19
# speedup vs baseline: 1.7082x; 1.0593x over previous
"""Mixed causal attention (B=8,L=1024,D=1024,H=16,NS=8) on 8 TRN2 cores.

Sharding: data-parallel over batch (core b owns batch b).  The per-position
(ns) projection weights are sharded by position: core c computes q/k/v for
position 1016+c for ALL batches, an AllGather distributes the results, and
each core extracts its batch's 8 tail rows with a one-hot selection matmul.

Pipeline (engine-balanced, keeps PE dense so HAM stays at full clock):
  A: V projection (psum-accumulated, strided interleave copy into vb)
  B: ns projections (6 psum banks accumulate over dt) -> AllGather (async)
  C: per head-pair i: QK proj strip et=i overlapped with g0 attention of
     pair i-1 (scores row-tiled 2-up, full-width exps, batched causal-mask
     multiply, fast reciprocal, fp32r broadcast matmul)
  T: tail merge (q/k/v rows 1016..1023 from gathered ns results)
  D: per pair i: g1 scores of pair i overlapped with AV of pair i-1 and
     one out-projection chunk (lt<4 columns complete after C)
  E: remaining out-projection chunks
"""

import sys
from contextlib import ExitStack

import numpy as np

sys.path.insert(0, "/opt/trn_rl_repo")

import ml_dtypes  # noqa: E402
import concourse.bass as bass  # noqa: E402
import concourse.tile as tile  # noqa: E402
from concourse import bacc, mybir  # noqa: E402
from concourse._compat import with_exitstack  # noqa: E402
from concourse.bass_utils import run_bass_kernel_spmd  # noqa: E402

B, L, D, H, NS = 8, 1024, 1024, 16, 8
HD = D // H          # 64
LS = L - NS          # 1016
NCORES = 8
NEG = -1.0e9
BF = mybir.dt.bfloat16
F32 = mybir.dt.float32
F32R = mybir.dt.float32r

_CACHE = {}
TRACE = False
DEBUG = False


@with_exitstack
def _attention_kernel(ctx: ExitStack, tc: tile.TileContext, aps: dict):
    nc = tc.nc
    Exp = mybir.ActivationFunctionType.Exp
    MUL = mybir.AluOpType.mult

    sb = ctx.enter_context(tc.tile_pool(name="persist", bufs=1))
    wpool = ctx.enter_context(tc.tile_pool(name="wpool", bufs=1))
    wns_pool = ctx.enter_context(tc.tile_pool(name="wns", bufs=2))
    stage = ctx.enter_context(tc.tile_pool(name="stage", bufs=3))
    accp = ctx.enter_context(tc.tile_pool(name="accp", bufs=2, space="PSUM"))
    dbgp = ctx.enter_context(tc.tile_pool(name="dbgp", bufs=1)) if DEBUG else None
    dram = ctx.enter_context(tc.tile_pool(name="dram", bufs=2, space="DRAM"))

    # ---- persistent SBUF tensors ----
    xT = sb.tile([128, 8 * 1024], BF)      # [d-part, dt*1024 + l]
    qT = sb.tile([128, 8 * 1024], BF)      # [e-part, et*1024 + l]
    kT = sb.tile([128, 8 * 1024], BF)
    vb = sb.tile([128, 8 * 1040], BF)      # [l-part, lt*1040 + h*65 + eh]; col h*65+64 = ones
    den2 = sb.tile([33, 512], F32)
    rec2 = sb.tile([33, 512], F32)
    oT = sb.tile([128, 8 * 1024], BF)      # [e-part, et*1024 + l]
    xtails = sb.tile([128, 64], BF)        # [d-part, dt*8 + bb]
    sel = sb.tile([64, 8], BF)             # one-hot row selector (per-core data)
    mbias = sb.tile([128, 8], F32)         # key-padding additive bias per k-block
    tri4 = sb.tile([128, 512], BF)         # 4x repeated [128,128] lower-tri 0/1

    fullg = sb.tile([64, 3072], BF)        # gathered: row n*8+bb
    # per-head exp'd score tiles (bufs=1; subtile deps pipeline across pairs)
    pt0 = [sb.tile([128, 2560], BF, name=f"pt0_{i}") for i in range(2)]  # g=0
    pt1 = [sb.tile([128, 4608], BF, name=f"pt1_{i}") for i in range(2)]  # g=1

    wq = sb.tile([128, 8 * 1024], BF)
    wk = sb.tile([128, 8 * 1024], BF)
    wv = wpool.tile([128, 8 * 1024], BF, name="wslot")

    # ---- input DMAs (batched 3D APs, spread across engine queues) ----
    def load_w(eng, dst, name):
        d = dst[:].rearrange("p (t l) -> p t l", l=1024)
        s = aps[name][:, :].rearrange("(t p) l -> p t l", p=128)
        eng.dma_start(d, s)

    load_w(nc.sync, xT, "xT")
    load_w(nc.scalar, wv, "wvT")
    nc.gpsimd.dma_start(xtails[:], aps["xtails"][:, :])
    nc.gpsimd.dma_start(sel[:], aps["sel"][:])
    nc.sync.dma_start(mbias[:], aps["maskbias"][:])
    nc.sync.dma_start(tri4[:], aps["tri4"][:])
    load_w(nc.sync, wq, "wqT")
    load_w(nc.scalar, wk, "wkT")

    nc.vector.memset(vb[:], 0.0)
    nc.vector.memset(
        vb[:].rearrange("p (t h c) -> p t h c", h=16, c=65)[:, :, :, 64:65], 1.0
    )
    nc.vector.memset(den2[:], 1.0)

    # ---- phase A: V projection, natural [l, e] layout interleaved per head ----
    def v_proj(eg):
        for lt in range(8):
            acc = accp.tile([128, 512], F32, name="acc")
            for dt in range(8):
                nc.tensor.matmul(
                    acc[:],
                    xT[:, dt * 1024 + lt * 128: dt * 1024 + lt * 128 + 128],
                    wv[:, dt * 1024 + eg * 512: dt * 1024 + eg * 512 + 512],
                    start=(dt == 0),
                    stop=(dt == 7),
                )
            base = lt * 1040 + eg * 520
            dst = vb[:, base: base + 520].rearrange("p (h c) -> p h c", c=65)[:, :, 0:64]
            src = acc[:].rearrange("p (h c) -> p h c", c=64)
            nc.vector.tensor_copy(dst, src)

    # ---- phase B: ns projections for my position (all batches) ----
    with tc.tile_pool(name="nsps", bufs=1, space="PSUM") as nsps:
        nspp = [nsps.tile([8, 512], F32, name=f"nsp{ck}") for ck in range(6)]
        for dt in range(8):
            wt = wns_pool.tile([128, 3072], BF)
            nc.gpsimd.dma_start(wt[:], aps["wnsT"][dt * 128:dt * 128 + 128, :])
            for ck in range(6):
                nc.tensor.matmul(
                    nspp[ck][:],
                    xtails[:, bass.ts(dt, 8)],
                    wt[:, bass.ts(ck, 512)],
                    start=(dt == 0),
                    stop=(dt == 7),
                )
        nsb = wns_pool.tile([8, 3072], BF, name="wt")
        for ck in range(6):
            nc.vector.tensor_copy(nsb[:, bass.ts(ck, 512)], nspp[ck][:])

    gin = dram.tile([8, 3072], BF)
    gout = dram.tile([64, 3072], BF)
    nc.gpsimd.dma_start(gin[:], nsb[:])
    nc.gpsimd.collective_compute(
        "AllGather",
        mybir.AluOpType.bypass,
        replica_groups=[list(range(NCORES))],
        ins=[gin.opt()],
        outs=[gout.opt()],
    )
    nc.gpsimd.dma_start(fullg[:], gout[:])

    for t in pt0 + pt1:
        nc.gpsimd.memset(t[:], 0.0)

    v_proj(0)
    v_proj(1)

    # wo reuses wv's buffer (pool bufs=3); DMA starts once V is done with wv
    wo = wpool.tile([128, 8 * 1024], BF, name="wslot")
    load_w(nc.gpsimd, wo, "woutT")

    spp = ctx.enter_context(tc.tile_pool(name="spp", bufs=4, space="PSUM"))
    opp = ctx.enter_context(tc.tile_pool(name="opp", bufs=2, space="PSUM"))

    def qk_proj(et):
        for w, dst in ((wq, qT), (wk, kT)):
            for lg in range(2):
                acc = accp.tile([128, 512], F32, name="acc")
                for dt in range(8):
                    nc.tensor.matmul(
                        acc[:],
                        w[:, dt * 1024 + et * 128: dt * 1024 + et * 128 + 128],
                        xT[:, dt * 1024 + lg * 512: dt * 1024 + lg * 512 + 512],
                        start=(dt == 0),
                        stop=(dt == 7),
                    )
                nc.vector.tensor_copy(
                    dst[:, et * 1024 + lg * 512: et * 1024 + lg * 512 + 512], acc[:]
                )

    def attn(et, g):
        """Scores (row-tiled 2-up) + exp + mask + AV + normalize for pair et."""
        nj = 4 * g + 4
        pts = pt0 if g == 0 else pt1
        qs = et * 1024 + g * 512
        # scores + exp, both heads interleaved per j (concurrent row tiles)
        sps = {}
        for j in range(nj):
            lead = max(0, j * 128 - g * 512)
            for half in range(2):
                r0 = half * 64
                sp = spp.tile([128, 512], F32, name="sp")
                nc.tensor.matmul(
                    sp[:, lead:512],
                    kT[r0:r0 + 64, et * 1024 + j * 128: et * 1024 + j * 128 + 128],
                    qT[r0:r0 + 64, qs + lead: qs + 512],
                    start=True,
                    stop=True,
                )
                sps[(j, half)] = sp
        for j in range(nj):
            lead = max(0, j * 128 - g * 512)
            for half in range(2):
                nc.scalar.activation(
                    pts[half][:, j * 512 + lead: j * 512 + 512],
                    sps[(j, half)][:, lead:512],
                    Exp,
                    bias=mbias[:, j: j + 1],
                    scale=0.125,
                )
        # causal mask: zero strictly-upper part of the 4 diagonal 128-blocks
        # (windows at moff + 640*w, batched in one strided op per head)
        moff = 2048 * g
        t3 = tri4[:].rearrange("p (w c) -> p w c", c=128)
        for half in range(2):
            w3 = (
                pts[half][:, moff: moff + 2560]
                .rearrange("p (w c) -> p w c", c=640)[:, :, 0:128]
            )
            nc.vector.tensor_tensor(w3, w3, t3, MUL)
        # AV (ones column in vb row 64 gives the softmax denominator)
        ops = []
        for half in range(2):
            h = 2 * et + half
            op = opp.tile([128, 512], F32, name="op")
            for j in range(nj):
                lead = max(0, j * 128 - g * 512)
                nc.tensor.matmul(
                    op[0:65, lead:512],
                    vb[:, j * 1040 + h * 65: j * 1040 + h * 65 + 65],
                    pts[half][:, j * 512 + lead: j * 512 + 512],
                    start=(j == 0),
                    stop=(j == nj - 1),
                )
            ops.append(op)
        nc.vector.tensor_copy(den2[0:1, :], ops[0][64:65, :])
        nc.vector.tensor_copy(den2[32:33, :], ops[1][64:65, :])
        nc.vector.reciprocal_approx_fast(rec2[:], den2[:])
        bbs = []
        for half in range(2):
            r16 = stage.tile([1, 512], BF, name="r16", bufs=2)
            nc.vector.tensor_copy(r16[:], rec2[32 * half: 32 * half + 1, :])
            bb = stage.tile([64, 512], BF, name="bb", bufs=2)
            nc.gpsimd.partition_broadcast(bb[:], r16[:])
            bbs.append(bb)
        if DEBUG and et == 0 and g == 0:
            dop = dbgp.tile([65, 512], F32, name="dop")
            nc.vector.tensor_copy(dop[:], ops[0][0:65, :])
            nc.gpsimd.dma_start(aps["dbg_op"][:, :], dop[:])
            nc.gpsimd.dma_start(aps["dbg_bb"][:, :], bbs[0][:])
        for half in range(2):
            nc.vector.tensor_tensor(
                oT[half * 64: half * 64 + 64, qs: qs + 512],
                ops[half][0:64, :],
                bbs[half][:],
                MUL,
            )

    def out_proj(lt, eg):
        acc = accp.tile([128, 512], F32, name="acc")
        for et in range(8):
            nc.tensor.matmul(
                acc[:],
                oT[:, et * 1024 + lt * 128: et * 1024 + lt * 128 + 128],
                wo[:, et * 1024 + eg * 512: et * 1024 + eg * 512 + 512],
                start=(et == 0),
                stop=(et == 7),
            )
        ys = stage.tile([128, 512], F32, name="ys")
        nc.vector.tensor_copy(ys[:], acc[:])
        nc.gpsimd.dma_start(
            aps["y"][lt * 128: lt * 128 + 128, eg * 512: eg * 512 + 512], ys[:]
        )

    def tailmerge(et):
        for c2 in (et, 8 + et):
            tp = spp.tile([128, 512], F32, name="sp")
            nc.tensor.matmul(
                tp[:, 0:8],
                fullg[:, c2 * 128: c2 * 128 + 128],
                sel[:],
                start=True,
                stop=True,
            )
            dst = qT if c2 < 8 else kT
            nc.vector.tensor_copy(
                dst[:, et * 1024 + 1016: et * 1024 + 1024], tp[:, 0:8]
            )

    def vb_tails():
        for vg in range(2):
            tp = spp.tile([128, 512], F32, name="sp")
            nc.tensor.matmul(
                tp[0:8, :],
                sel[:],
                fullg[:, 2048 + vg * 512: 2048 + vg * 512 + 512],
                start=True,
                stop=True,
            )
            vt = stage.tile([8, 512], BF, name="vt", bufs=1)
            nc.vector.tensor_copy(vt[:], tp[0:8, :])
            dst = (
                vb[120:128, 7 * 1040 + vg * 520: 7 * 1040 + vg * 520 + 520]
                .rearrange("p (h c) -> p h c", c=65)[:, :, 0:64]
            )
            nc.gpsimd.dma_start(dst, vt[:].rearrange("p (h c) -> p h c", c=64))

    # ---- unified pipeline: QK(i) | g0+tails(i-1) | g1(i-2) ----
    for i in range(10):
        if i < 8:
            qk_proj(i)
        if 1 <= i <= 8:
            attn(i - 1, 0)
            tailmerge(i - 1)
            if i == 1:
                vb_tails()
        if 2 <= i <= 9:
            attn(i - 2, 1)
        if i == 9:
            for lt in range(4):
                for eg in range(2):
                    out_proj(lt, eg)

    # ---- remaining output projection (needs all g1 done) ----
    for lt in range(4, 8):
        for eg in range(2):
            out_proj(lt, eg)

    if DEBUG:
        nc.gpsimd.dma_start(aps["dbg_qT"][:, :], qT[:])
        nc.gpsimd.dma_start(aps["dbg_kT"][:, :], kT[:])
        nc.gpsimd.dma_start(aps["dbg_vb"][:, :], vb[:])
        nc.gpsimd.dma_start(aps["dbg_oT"][:, :], oT[:])
        nc.gpsimd.dma_start(aps["dbg_pt0"][:, :], pt0[0][:, 0:2048])


def _build():
    if "nc" in _CACHE:
        return _CACHE["nc"]
    nc = bacc.Bacc("TRN2", target_bir_lowering=False, debug=False, num_devices=NCORES)
    aps = {}
    for name, shape, dt in [
        ("xT", [1024, 1024], BF),
        ("wqT", [1024, 1024], BF),
        ("wkT", [1024, 1024], BF),
        ("wvT", [1024, 1024], BF),
        ("woutT", [1024, 1024], BF),
        ("wnsT", [1024, 3072], BF),
        ("xtails", [128, 64], BF),
        ("sel", [64, 8], BF),
        ("maskbias", [128, 8], F32),
        ("tri4", [128, 512], BF),
    ]:
        aps[name] = nc.dram_tensor(name, shape, dt, kind="ExternalInput").ap()
    aps["y"] = nc.dram_tensor("y", [1024, 1024], F32, kind="ExternalOutput").ap()
    if DEBUG:
        for name, shape, dt in [
            ("dbg_op", [65, 512], F32),
            ("dbg_bb", [64, 512], BF),
            ("dbg_qT", [128, 8192], BF),
            ("dbg_kT", [128, 8192], BF),
            ("dbg_vb", [128, 8320], BF),
            ("dbg_oT", [128, 8192], BF),
            ("dbg_pt0", [128, 2048], BF),
        ]:
            aps[name] = nc.dram_tensor(name, shape, dt, kind="ExternalOutput").ap()

    with tile.TileContext(nc) as tc:
        _attention_kernel(tc, aps)
    nc.compile()
    _CACHE["nc"] = nc
    return nc


def kernel(x, key_padding_mask, Wq_s, Wk_s, Wv_s, Wq_ns, Wk_ns, Wv_ns, W_out, **kw):
    x = np.asarray(x, np.float32)
    mask = np.asarray(key_padding_mask)
    bf = ml_dtypes.bfloat16

    wqT = np.ascontiguousarray(np.asarray(Wq_s, np.float32).T.astype(bf))
    wkT = np.ascontiguousarray(np.asarray(Wk_s, np.float32).T.astype(bf))
    wvT = np.ascontiguousarray(np.asarray(Wv_s, np.float32).T.astype(bf))
    woT = np.ascontiguousarray(np.asarray(W_out, np.float32).T.astype(bf))

    # tri4: 4 copies of the [128,128] lower-triangular keep-mask (p <= c)
    p = np.arange(128)[:, None]
    c_ = np.arange(128)[None, :]
    tri01 = (p <= c_).astype(bf)
    tri4 = np.ascontiguousarray(np.tile(tri01, (1, 4)))

    Wq_ns = np.asarray(Wq_ns, np.float32)
    Wk_ns = np.asarray(Wk_ns, np.float32)
    Wv_ns = np.asarray(Wv_ns, np.float32)

    in_maps = []
    for c in range(NCORES):
        xT = np.ascontiguousarray(x[c].T.astype(bf))
        xt_full = x[:, LS + c, :].T.astype(bf)  # [1024 d, 8 bb]
        xtails = np.ascontiguousarray(
            xt_full.reshape(8, 128, 8).transpose(1, 0, 2).reshape(128, 64)
        )
        wnsT = np.ascontiguousarray(
            np.concatenate([Wq_ns[c].T, Wk_ns[c].T, Wv_ns[c].T], axis=1).astype(bf)
        )
        selm = np.zeros((64, 8), bf)
        for n in range(NS):
            selm[n * 8 + c, n] = 1.0
        mb = np.where(mask[c], 0.0, NEG).astype(np.float32).reshape(8, 128).T
        mb = np.ascontiguousarray(mb)
        in_maps.append(
            {
                "xT": xT,
                "wqT": wqT,
                "wkT": wkT,
                "wvT": wvT,
                "woutT": woT,
                "wnsT": wnsT,
                "xtails": xtails,
                "sel": selm,
                "maskbias": mb,
                "tri4": tri4,
            }
        )

    nc = _build()
    res = run_bass_kernel_spmd(nc, in_maps, list(range(NCORES)), trace=TRACE)
    _CACHE["exec_time_ns"] = res.exec_time_ns
    _CACHE["res"] = res
    out = np.stack([res.results[c]["y"] for c in range(NCORES)], axis=0)
    return out.astype(np.float32)
